# revision 1
# baseline (speedup 1.0000x reference)
"""Trainium2 Bass kernel for the gated-attention layer.

Sharding: 8 cores = (2 batches) x (4 head-groups of 4 heads each).
Core c handles batch b = c // 4, heads 4*(c%4) .. 4*(c%4)+4 (d_model cols
256*(c%4) .. +256).  Each core computes
    y_c = gate (.) (V_heads @ Wo_rows)  +  (1/4)[gate (.) bo + (1-gate) (.) VG]
for its full batch [2048, 1024]; the host sums the 4 partials per batch.

All large matmuls run in bf16 (fp32 PSUM accumulation).  Softmax is computed
without max-subtraction (scores*0.125 ~ N(0,1)) as exp on ScalarE during the
PSUM->SBUF evacuation; the denominator comes from a ones-column appended to V
in the A@V matmul, and the per-row division (together with the gate) is folded
into the V^T normalization before the output projection.
"""

import sys

for _p in ("/root/.axon_site/_ro/trn_rl_repo", "/opt/trn_rl_repo"):
    if _p not in sys.path:
        sys.path.append(_p)

import numpy as np
import ml_dtypes

B, L, D, H = 2, 2048, 1024, 16
E = D // H          # 64, head dim
N_CORES = 8
HG = 4              # heads per core
CW = HG * E         # 256, column width per core
KT_TILES = D // 128  # 8 contraction chunks
LT = L // 128        # 16 l_tiles / s_tiles
LCHUNK = 1024        # l-chunk for the attention inner loop
NLC = L // LCHUNK    # 2

BF16 = ml_dtypes.bfloat16

_CACHED = {}


def _patch_drain(tile_mod, mybir):
    """This walrus build only accepts one sync-wait on a Drain; spread the
    final Tile drain's waits over single-wait NOPs."""
    from concourse.vector_clock import ScopedClock

    def _dab(self, tick_clock, wait_clock):
        nc = self.nc
        drain_inst = nc.sync.drain()
        wait_clock.add_sem_waits(
            drain_inst.ins, ScopedClock({None: tick_clock.global_clock})
        )
        waits = list(drain_inst.ins.sync_info.on_wait)
        if len(waits) > 1:
            drain_inst.ins.sync_info.on_wait = waits[:1]
            for w in waits[1:]:
                nop = nc.sync.nop()
                if nop.ins.sync_info is None:
                    nop.ins.sync_info = mybir.SyncInfo(on_wait=[w], on_update=[])
                else:
                    nop.ins.sync_info.on_wait = [w]
        nc.all_engine_barrier()
        assert self.sems is not None
        popped = nc._tile_sem_poison_stack.pop()
        assert popped is self._sem_poison
        nc.clear_and_free_semaphores(list(self.sems.allocated().values()))
        nc.all_engine_barrier()

    tile_mod.TileContext._drain_and_barrier = _dab


def _emit(nc, tile, mybir, ctx, tc, t):
    """Emit the per-core program. t = dict of dram APs."""
    f32 = mybir.dt.float32
    bf16 = mybir.dt.bfloat16
    AF = mybir.ActivationFunctionType
    X = mybir.AxisListType.X
    SCALE = 1.0 / np.sqrt(E)

    consts = ctx.enter_context(tc.tile_pool(name="consts", bufs=1))

    # ---- load inputs to SBUF ----
    cb = consts.tile([128, CW + 5], f32)
    nc.sync.dma_start(out=cb, in_=t["cb"])
    bq, bk = cb[:, 0:2], cb[:, 2:4]
    bf_b = cb[:, 4:5]
    bv_b = cb[:, 5:5 + CW]
    bo4 = consts.tile([1, D], bf16)
    nc.sync.dma_start(out=bo4, in_=t["bo4"])
    bg4 = consts.tile([1, D], f32)
    nc.sync.dma_start(out=bg4, in_=t["bg4"])

    xT = [consts.tile([128, L], bf16, name=f"xT{k}", tag=f"xT{k}") for k in range(KT_TILES)]
    xTd = t["xT"].rearrange("(t p) l -> t p l", p=128)

    def w_tiles(name, cols):
        tiles = [consts.tile([128, cols], bf16, name=f"{name}{k}", tag=f"{name}{k}") for k in range(KT_TILES)]
        return tiles, t[name].rearrange("(t p) c -> t p c", p=128)

    wq, wqd = w_tiles("wq", CW)
    wk, wkd = w_tiles("wk", CW)
    wv, wvd = w_tiles("wv", CW + 1)
    wg, wgd = w_tiles("wg", D)
    # interleave loads so the k-th chunk of everything the first matmuls
    # need arrives together (first QT matmul can start after ~2 chunks)
    for k in range(KT_TILES):
        nc.sync.dma_start(out=xT[k], in_=xTd[k])
        nc.sync.dma_start(out=wq[k], in_=wqd[k])
        nc.sync.dma_start(out=wk[k], in_=wkd[k])
        nc.sync.dma_start(out=wv[k], in_=wvd[k])
    wo = [consts.tile([128, D], bf16, name=f"wo{k}", tag=f"wo{k}") for k in range(2)]
    wod = t["wo"].rearrange("(t p) c -> t p c", p=128)
    for k in range(2):
        nc.sync.dma_start(out=wo[k], in_=wod[k])
    for k in range(KT_TILES):
        nc.sync.dma_start(out=wg[k], in_=wgd[k])

    # ---- phase A: projections + gate + global-context ----
    qt = [consts.tile([128, L], bf16, name=f"qt{i}", tag=f"qt{i}") for i in range(2)]
    kt = [consts.tile([128, L], bf16, name=f"kt{i}", tag=f"kt{i}") for i in range(2)]
    v_aug = [consts.tile([128, HG * (E + 1)], bf16, name=f"vaug{i}", tag=f"vaug{i}") for i in range(LT)]
    import concourse.bass as bass_mod
    dramp = ctx.enter_context(tc.tile_pool(name="dramp", bufs=2, space="DRAM"))
    gate_f = consts.tile([1, L], f32)
    gate_t = consts.tile([128, LT], f32)
    gateomg = consts.tile([2, L], bf16)
    bovg = consts.tile([2, D], bf16)
    gate_b = consts.tile([1, L], bf16)
    omg_b = consts.tile([1, L], bf16)
    vg4_b = consts.tile([1, D], bf16)
    xsum = consts.tile([128, KT_TILES], f32)
    xsum_b = consts.tile([128, KT_TILES], bf16)

    with tc.tile_pool(name="pa_psum", bufs=4, space="PSUM") as pa, \
         tc.tile_pool(name="rows_psum", bufs=1, space="PSUM") as rows, \
         tc.tile_pool(name="pa_sb", bufs=3) as pasb:
        # QT / KT, c=0 only (pair-0 inputs); c=1 is emitted between the
        # attention pairs so PE fills pair-0's ACT-bound slack
        def emit_qtkt(c, pool, tag):
            for dst, w, bias in ((qt, wq, bq), (kt, wk, bk)):
                for lo in range(0, L, 512):
                    ps = pool.tile([128, 512], f32, name="qk_t", tag=tag)
                    for k in range(KT_TILES):
                        nc.tensor.matmul(
                            out=ps, lhsT=w[k][:, c * 128:(c + 1) * 128],
                            rhs=xT[k][:, lo:lo + 512],
                            start=(k == 0), stop=(k == KT_TILES - 1))
                    nc.vector.tensor_scalar_add(
                        out=dst[c][:, lo:lo + 512], in0=ps, scalar1=bias[:, c:c + 1])
        emit_qtkt(0, pa, "pa")
        # V: [L, CW] rows (col CW = x@Wf gate pre-activation riding along);
        # store interleaved [64 v | 1.0] per head
        for s in range(LT):
            ps = pa.tile([128, CW + 1], f32, name="pav_t", tag="pa")
            for k in range(KT_TILES):
                nc.tensor.matmul(
                    out=ps, lhsT=xT[k][:, s * 128:(s + 1) * 128], rhs=wv[k],
                    start=(k == 0), stop=(k == KT_TILES - 1))
            va = v_aug[s]
            src = ps[:, 0:CW].rearrange("p (h c) -> p h c", c=E)
            dst = va.rearrange("p (h c) -> p h c", c=E + 1)[:, :, 0:E]
            nc.vector.tensor_add(dst, src, bv_b.rearrange("p (h c) -> p h c", c=E))
            for h in range(HG):
                nc.vector.memset(va[:, h * (E + 1) + E: h * (E + 1) + E + 1], 1.0)
            nc.scalar.activation(gate_t[:, s:s + 1], ps[:, CW:CW + 1], AF.Sigmoid,
                                 bias=bf_b[:, 0:1])
        # gate row layout [1, L] via DRAM bounce from gate_t
        gd2 = dramp.tile([1, L], f32, name="gd2_t", tag="gd2")
        nc.sync.dma_start(out=gd2, in_=gate_t)
        nc.sync.dma_start(out=gate_f, in_=bass_mod.AP(
            tensor=gd2.tensor, offset=gd2.offset, ap=[[0, 1], [1, LT], [LT, 128]]))
        nc.vector.tensor_copy(gate_b, gate_f)
        nc.vector.tensor_scalar(out=omg_b, in0=gate_f, scalar1=-1.0, scalar2=1.0,
                                op0=mybir.AluOpType.mult, op1=mybir.AluOpType.add)
        # [gate; 1-gate] stacked on partitions 0/1 for the fused bias matmul
        nc.sync.dma_start(out=gateomg[0:1, :], in_=gate_b)
        nc.sync.dma_start(out=gateomg[1:2, :], in_=omg_b)
        nc.sync.dma_start(out=bovg[0:1, :], in_=bo4)
        # global context row: VG4 = (mean_l x) @ Wg * 0.25 + bg/4
        for k in range(KT_TILES):
            nc.vector.reduce_sum(out=xsum[:, k:k + 1], in_=xT[k], axis=X)
        nc.vector.tensor_copy(xsum_b, xsum)

    # ---- phase B: attention per head ----
    vt = [consts.tile([128, L], bf16, name=f"vt{i}", tag=f"vt{i}") for i in range(2)]
    attn_sb = ctx.enter_context(tc.tile_pool(name="attn_sb", bufs=4))
    rowp = ctx.enter_context(tc.tile_pool(name="rowp", bufs=3))
    with tc.tile_pool(name="st_psum", bufs=2, space="PSUM") as stp, \
         tc.tile_pool(name="av_psum", bufs=2, space="PSUM") as avp:
        vg_sb = ctx.enter_context(tc.tile_pool(name="vg_sb", bufs=2))

        def emit_mid():
            # pair-1 inputs + global-context, using the churning score slots
            emit_qtkt(1, stp, "st")
            for do in range(0, D, 512):
                ps = stp.tile([1, 512], f32, name="vgp_t", tag="st")
                for k in range(KT_TILES):
                    nc.tensor.matmul(out=ps, lhsT=xsum_b[:, k:k + 1],
                                     rhs=wg[k][:, do:do + 512],
                                     start=(k == 0), stop=(k == KT_TILES - 1))
                sc = vg_sb.tile([1, 512], f32, name="vgrow_t", tag="vgrow")
                nc.scalar.activation(sc, ps, AF.Copy, scale=0.25 / L)
                nc.vector.tensor_add(vg4_b[:, do:do + 512], sc, bg4[:, do:do + 512])
            nc.sync.dma_start(out=bovg[1:2, :], in_=vg4_b)

        for pr in range(HG // 2):
            pair = (2 * pr, 2 * pr + 1)
            if pr == 1:
                emit_mid()
            for lc in range(NLC):
                lbase = lc * LCHUNK
                avs = {}
                for h in pair:
                    avs[h] = avp.tile([E + 1, LCHUNK], f32, name=f"av{h}", tag="av")
                for s in range(LT):
                    sts = {}
                    # score matmuls for both heads adjacent: they sit on
                    # different 64-row strips of the PE and run concurrently
                    for h in pair:
                        kpart = 64 * (h % 2)
                        st = stp.tile([128, LCHUNK], f32, name=f"st{h}", tag="st")
                        for o in range(0, LCHUNK, 512):
                            nc.tensor.matmul(
                                out=st[:, o:o + 512],
                                lhsT=kt[h // 2][kpart:kpart + 64, s * 128:(s + 1) * 128],
                                rhs=qt[h // 2][kpart:kpart + 64, lbase + o:lbase + o + 512],
                                start=True, stop=True)
                        sts[h] = st
                    for h in pair:
                        ex = attn_sb.tile([128, LCHUNK], bf16, name="ex_t", tag="ex")
                        nc.scalar.activation(ex, sts[h], AF.Exp, scale=SCALE)
                        va_lo = h * (E + 1)
                        for o in range(0, LCHUNK, 512):
                            nc.tensor.matmul(
                                out=avs[h][:, o:o + 512],
                                lhsT=v_aug[s][:, va_lo:va_lo + E + 1],
                                rhs=ex[:, o:o + 512],
                                start=(s == 0), stop=(s == LT - 1))
                # normalize + gate, into vt rows
                nlt = LCHUNK // 128   # l_tiles in this chunk
                for h in pair:
                    # evacuate the accumulator promptly so the PSUM slot frees
                    # for the next chunk; normalize entirely from SBUF
                    av = attn_sb.tile([E + 1, LCHUNK], f32, name=f"avc{h}", tag="avc")
                    nc.vector.tensor_copy(av, avs[h])
                    # denominator row -> [128, nlt] layout via DRAM bounce
                    rd = dramp.tile([1, LCHUNK], f32, name="rd_t", tag="rd")
                    nc.sync.dma_start(out=rd, in_=av[E:E + 1, :])
                    dT = attn_sb.tile([128, nlt], f32, name="dT_t", tag="dT")
                    nc.sync.dma_start(out=dT, in_=bass_mod.AP(
                        tensor=rd.tensor, offset=rd.offset, ap=[[1, 128], [128, nlt]]))
                    rc = attn_sb.tile([128, nlt], f32, name="rc_t", tag="rc")
                    nc.vector.reciprocal(rc, dT)
                    nc.vector.tensor_mul(rc, rc, gate_t[:, lc * nlt:(lc + 1) * nlt])
                    rd2 = dramp.tile([1, LCHUNK], f32, name="rd2_t", tag="rd2")
                    nc.sync.dma_start(
                        out=bass_mod.AP(tensor=rd2.tensor, offset=rd2.offset,
                                        ap=[[1, 128], [128, nlt]]),
                        in_=rc)
                    bc = attn_sb.tile([64, LCHUNK], f32, name="bc_t", tag="bc")
                    nc.sync.dma_start(out=bc, in_=bass_mod.AP(
                        tensor=rd2.tensor, offset=rd2.offset,
                        ap=[[0, 64], [1, LCHUNK]]))
                    vpart = 64 * (h % 2)
                    nc.vector.tensor_mul(
                        vt[h // 2][vpart:vpart + 64, lbase:lbase + LCHUNK],
                        av[0:E, :], bc)

    # ---- phase C: output projection + fusion terms ----
    out_sb = ctx.enter_context(tc.tile_pool(name="out_sb", bufs=3))
    with tc.tile_pool(name="op_psum", bufs=2, space="PSUM") as opp:
        yd = t["y"].rearrange("(t p) d -> t p d", p=128)
        for lt in range(LT):
            ps = opp.tile([128, D], f32, name="op_t", tag="op")
            lsl = slice(lt * 128, (lt + 1) * 128)
            for do in range(0, D, 512):
                for kc in range(2):
                    nc.tensor.matmul(out=ps[:, do:do + 512],
                                     lhsT=vt[kc][:, lsl],
                                     rhs=wo[kc][:, do:do + 512],
                                     start=(kc == 0), stop=False)
                nc.tensor.matmul(out=ps[:, do:do + 512],
                                 lhsT=gateomg[:, lsl], rhs=bovg[:, do:do + 512],
                                 start=False, stop=True)
            ot = out_sb.tile([128, D], f32, name="ot_t", tag="ot")
            nc.vector.tensor_copy(ot, ps)
            nc.sync.dma_start(out=yd[lt], in_=ot)


def _build():
    if "nc" in _CACHED:
        return _CACHED["nc"]
    import concourse.bass as bass
    import concourse.tile as tile
    from concourse import mybir
    from contextlib import ExitStack

    _patch_drain(tile, mybir)
    nc = bass.Bass("TRN2", target_bir_lowering=False, debug=False)
    f32, bf16 = mybir.dt.float32, mybir.dt.bfloat16
    t = {
        "xT": nc.dram_tensor("xT", [D, L], bf16, kind="ExternalInput").ap(),
        "cb": nc.dram_tensor("cb", [128, CW + 5], f32, kind="ExternalInput").ap(),
        "wq": nc.dram_tensor("wq", [D, CW], bf16, kind="ExternalInput").ap(),
        "wk": nc.dram_tensor("wk", [D, CW], bf16, kind="ExternalInput").ap(),
        "wv": nc.dram_tensor("wv", [D, CW + 1], bf16, kind="ExternalInput").ap(),
        "wo": nc.dram_tensor("wo", [CW, D], bf16, kind="ExternalInput").ap(),
        "wg": nc.dram_tensor("wg", [D, D], bf16, kind="ExternalInput").ap(),
        "bo4": nc.dram_tensor("bo4", [1, D], bf16, kind="ExternalInput").ap(),
        "bg4": nc.dram_tensor("bg4", [1, D], f32, kind="ExternalInput").ap(),
        "y": nc.dram_tensor("y", [L, D], f32, kind="ExternalOutput").ap(),
    }
    with tile.TileContext(nc) as tc:
        with ExitStack() as ctx:
            _emit(nc, tile, mybir, ctx, tc, t)
    _split_multi_waits(nc, mybir)
    _CACHED["nc"] = nc
    return nc


def _split_multi_waits(nc, mybir):
    """This walrus build encodes at most one sync-wait per instruction; move
    extra waits onto same-engine NOPs inserted right before the instruction."""
    ctr = 0
    for blk in nc.m.functions[0].blocks:
        insts = list(blk.instructions)
        out = []
        for inst in insts:
            si = getattr(inst, "sync_info", None)
            if si is not None and si.on_wait is not None and len(si.on_wait) > 1:
                waits = list(si.on_wait)
                for w in waits[:-1]:
                    nop = mybir.InstNoOp(
                        name=f"I-waitsplit-{ctr}",
                        engine=inst.engine,
                        sync_info=mybir.SyncInfo(on_wait=[w], on_update=[]),
                        bass_nofuse=True,
                    )
                    ctr += 1
                    out.append(nop)
                si.on_wait = waits[-1:]
            out.append(inst)
        if len(out) != len(insts):
            blk.instructions[:] = out


def _prep_core_inputs(c, inputs, bf_val, shared):
    b, g = c // 4, c % 4
    cols = slice(g * CW, (g + 1) * CW)
    m = {
        "xT": shared["xT"][b],
        "wq": np.ascontiguousarray(inputs["Wq"][:, cols]).astype(BF16),
        "wk": np.ascontiguousarray(inputs["Wk"][:, cols]).astype(BF16),
        "wv": np.ascontiguousarray(np.concatenate(
            [inputs["Wv"][:, cols], inputs["Wf"]], axis=1)).astype(BF16),
        "wo": np.ascontiguousarray(inputs["Wo"][cols, :]).astype(BF16),
        "wg": shared["wg"],
        "cb": np.concatenate([
            inputs["bq"][cols].reshape(2, 128).T,
            inputs["bk"][cols].reshape(2, 128).T,
            np.full((128, 1), bf_val, np.float32),
            np.broadcast_to(inputs["bv"][cols][None, :], (128, CW)),
        ], axis=1).astype(np.float32),
        "bo4": (inputs["bo"][None, :] * 0.25).astype(BF16),
        "bg4": (inputs["bg"][None, :] * 0.25).astype(np.float32),
    }
    return m


def kernel(**inputs):
    from concourse import bass_utils

    bf_val = float(np.asarray(inputs["bf"]).reshape(-1)[0])
    nc = _build()
    shared = {
        "xT": [np.ascontiguousarray(inputs["x"][b].T).astype(BF16)
               for b in range(B)],
        "wg": inputs["Wg"].astype(BF16),
    }
    in_maps = [_prep_core_inputs(c, inputs, bf_val, shared) for c in range(N_CORES)]
    res = bass_utils.run_bass_kernel_spmd(nc, in_maps, core_ids=list(range(N_CORES)))
    out = np.zeros((B, L, D), np.float32)
    for c in range(N_CORES):
        out[c // 4] += res.results[c]["y"]
    return out



# revision 2
# speedup vs baseline: 1.0442x; 1.0442x over previous
"""Trainium2 Bass kernel v2 for the gated-attention layer.

Sharding: 8 cores = (2 batches) x (4 head-groups of 4 heads each), as v1.

Key structure changes vs v1:
- Flipped AV matmul: out[q, head_dim] = ex_tile.T @ v_slice with N=65 per
  matmul (cost-model charges by output free size) -> AV cost halves, and
  denominators land per-partition (no DMA-bounce transposes).
- VG and gate pre-activations via N=1 matmuls (out [128,1] per k-tile).
- exp split between ScalarE (exact) and DVE (Schraudolph int16->bf16
  bitcast) so ACT is not the attention bottleneck.
- chunk-major loop (512 queries) with interleaved emission so PE never
  head-of-line blocks on exp.
- Output projection per chunk; y evacuated on ACT, DMA'd per 512-col tile.
"""

import sys

for _p in ("/root/.axon_site/_ro/trn_rl_repo", "/opt/trn_rl_repo"):
    if _p not in sys.path:
        sys.path.append(_p)

import numpy as np
import ml_dtypes

B, L, D, H = 2, 2048, 1024, 16
E = D // H          # 64 head dim
N_CORES = 8
HG = 4              # heads per core
CW = HG * E         # 256 cols per core
KT = D // 128       # 8 contraction k-tiles
NS = L // 128       # 16 key tiles
NCH = 4             # query chunks
CHW = L // NCH      # 512 queries per chunk
NQB = CHW // 128    # 4 q-blocks per chunk
SCALE = 1.0 / np.sqrt(E)

# Schraudolph exp on DVE for these key-tiles (rest on ACT); heads 0/1 are
# scored inside the tail window where DVE also runs norm/transpose-evac,
# so they get fewer DVE tiles.
DVE_KT_BY_H = {
    0: (1, 3, 5, 7, 9, 11, 13),
    1: (2, 4, 6, 8, 10, 12, 14),
    2: (0, 2, 4, 6, 8, 10, 12, 14, 15),
    3: (1, 3, 5, 7, 9, 11, 13, 15, 0),
}
SCH_A = 128.0 / np.log(2.0) * SCALE    # 23.083
SCH_B = 16251.0                        # 127*128 - 5.5 (centering) + 0.5 (trunc)

BF16 = ml_dtypes.bfloat16

_CACHED = {}
SPLIT_WAITS = True  # walrus HW build needs single-wait insts; CoreSim chokes on the split NoOps


def _patch_drain(tile_mod, mybir):
    """This walrus build only accepts one sync-wait on a Drain; spread the
    final Tile drain's waits over single-wait NOPs."""
    from concourse.vector_clock import ScopedClock

    def _dab(self, tick_clock, wait_clock):
        nc = self.nc
        drain_inst = nc.sync.drain()
        wait_clock.add_sem_waits(
            drain_inst.ins, ScopedClock({None: tick_clock.global_clock})
        )
        waits = list(drain_inst.ins.sync_info.on_wait)
        if len(waits) > 1:
            drain_inst.ins.sync_info.on_wait = waits[:1]
            for w in waits[1:]:
                nop = nc.sync.nop()
                if nop.ins.sync_info is None:
                    nop.ins.sync_info = mybir.SyncInfo(on_wait=[w], on_update=[])
                else:
                    nop.ins.sync_info.on_wait = [w]
        nc.all_engine_barrier()
        assert self.sems is not None
        popped = nc._tile_sem_poison_stack.pop()
        assert popped is self._sem_poison
        nc.clear_and_free_semaphores(list(self.sems.allocated().values()))
        nc.all_engine_barrier()

    tile_mod.TileContext._drain_and_barrier = _dab


def _split_multi_waits(nc, mybir):
    """One sync-wait per instruction; move extras onto same-engine NOPs."""
    ctr = 0
    for blk in nc.m.functions[0].blocks:
        insts = list(blk.instructions)
        out = []
        for inst in insts:
            si = getattr(inst, "sync_info", None)
            if si is not None and si.on_wait is not None and len(si.on_wait) > 1:
                waits = list(si.on_wait)
                for w in waits[:-1]:
                    nop = mybir.InstNoOp(
                        name=f"I-waitsplit-{ctr}",
                        engine=inst.engine,
                        sync_info=mybir.SyncInfo(on_wait=[w], on_update=[]),
                        bass_nofuse=True,
                    )
                    ctr += 1
                    out.append(nop)
                si.on_wait = waits[-1:]
            out.append(inst)
        if len(out) != len(insts):
            blk.instructions[:] = out


def _emit(nc, tile, mybir, ctx, tc, t):
    import concourse.bass as bass_mod

    f32 = mybir.dt.float32
    bf16 = mybir.dt.bfloat16
    i16 = mybir.dt.int16
    AF = mybir.ActivationFunctionType
    OP = mybir.AluOpType
    X = mybir.AxisListType.X

    consts = ctx.enter_context(tc.tile_pool(name="consts", bufs=1))
    dramp = ctx.enter_context(tc.tile_pool(name="dramp", bufs=2, space="DRAM"))

    # ---- SBUF constants / persistent tiles ----
    cb = consts.tile([128, 5 + CW], f32)
    nc.sync.dma_start(out=cb, in_=t["cb"])
    bq_t, bk_t = cb[:, 0:2], cb[:, 2:4]
    bf_c = cb[:, 4:5]
    bv_b = cb[:, 5:5 + CW]

    # all host-side tensors are packed [128, KT*cols] (k-tiles side by side)
    xT_all = consts.tile([128, KT * L], bf16, name="xT_all", tag="xT_all")
    xT = [xT_all[:, k * L:(k + 1) * L] for k in range(KT)]

    def w_packed(name, cols):
        all_t = consts.tile([128, KT * cols], bf16, name=f"{name}_all",
                            tag=f"{name}_all")
        return all_t, [all_t[:, k * cols:(k + 1) * cols] for k in range(KT)]

    wq_all, wq = w_packed("wq", CW)
    wk_all, wk = w_packed("wk", CW)
    wv_all, wv = w_packed("wv", CW)
    wg_all, wg = w_packed("wg", D)
    wfb = consts.tile([128, KT], bf16)
    ident = consts.tile([128, 128], bf16)
    bo4 = consts.tile([1, D], bf16)
    bgT = consts.tile([128, KT], f32)
    wo_all = consts.tile([128, 2 * D], bf16, name="wo_all", tag="wo_all")
    wo = [wo_all[:, k * D:(k + 1) * D] for k in range(2)]

    # few, large DMAs (HWDGE issue is a serialized ~630ns/DMA resource):
    # wv + wfb first, xT in 4 big 2-ktile pieces, then wq/wk, wo, wg.
    nc.sync.dma_start(out=wv_all, in_=t["wv"])
    nc.scalar.dma_start(out=wfb, in_=t["wfb"])
    for kp in range(KT // 2):
        nc.sync.dma_start(out=xT_all[:, 2 * kp * L:(2 * kp + 2) * L],
                          in_=t["xT"][:, 2 * kp * L:(2 * kp + 2) * L])
    nc.scalar.dma_start(out=wq_all, in_=t["wq"])
    nc.scalar.dma_start(out=wk_all, in_=t["wk"])
    nc.sync.dma_start(out=wo_all, in_=t["wo"])
    nc.scalar.dma_start(out=ident, in_=t["ident"])
    nc.scalar.dma_start(out=bo4, in_=t["bo4"])
    nc.scalar.dma_start(out=bgT, in_=t["bgT"])
    for half in range(2):
        nc.sync.dma_start(out=wg_all[:, half * 4 * D:(half + 1) * 4 * D],
                          in_=t["wg"][:, half * 4 * D:(half + 1) * 4 * D])

    qt = [consts.tile([128, L], bf16, name=f"qt{i}", tag=f"qt{i}") for i in range(2)]
    kt_ = [consts.tile([128, L], bf16, name=f"kt{i}", tag=f"kt{i}") for i in range(2)]
    v_aug = [consts.tile([128, CW], bf16, name=f"va{i}", tag=f"va{i}")
             for i in range(NS)]
    ones_c = consts.tile([128, 1], bf16)
    vt = [consts.tile([128, L], bf16, name=f"vt{i}", tag=f"vt{i}") for i in range(2)]
    gate_t = consts.tile([128, NS], f32)
    gate_b = consts.tile([128, NS], bf16)
    omg_b = consts.tile([128, NS], bf16)
    gateomg = consts.tile([2, L], bf16)
    bovg = consts.tile([2, D], bf16)
    xsum = consts.tile([128, KT], f32)
    xsum_b = consts.tile([128, KT], bf16)
    vg4T = consts.tile([128, KT], bf16)

    nc.vector.memset(ones_c, 1.0)

    # ---- phase A: projections, gate, global context ----
    import os as _os0
    _cut = int(_os0.environ.get("K2_CUT", "99"))
    if _cut == 0:
        return
    with tc.tile_pool(name="pav", bufs=2, space="PSUM") as pav, \
         tc.tile_pool(name="paq", bufs=2, space="PSUM") as paq, \
         tc.tile_pool(name="pag", bufs=1, space="PSUM") as pag:
        # V projection + per-tile evac with bias
        _nv = int(_os0.environ.get("K2_NV", str(NS)))
        _noevac = _os0.environ.get("K2_NOEVAC")
        for s in range(_nv):
            ps = pav.tile([128, CW], f32, name="vps", tag="vps")
            for k in range(KT):
                nc.tensor.matmul(out=ps, lhsT=xT[k][:, s * 128:(s + 1) * 128],
                                 rhs=wv[k], start=(k == 0), stop=(k == KT - 1))
            if not _noevac:
                nc.vector.tensor_add(v_aug[s], ps, bv_b)
        if _cut <= 1:
            return
        # gate pre-activations via N=1 matmuls, one sigmoid
        gpre = pag.tile([128, NS], f32, name="gpre", tag="gpre")
        for s in range(NS):
            for k in range(KT):
                nc.tensor.matmul(out=gpre[:, s:s + 1],
                                 lhsT=xT[k][:, s * 128:(s + 1) * 128],
                                 rhs=wfb[:, k:k + 1],
                                 start=(k == 0), stop=(k == KT - 1))
        nc.scalar.activation(gate_t, gpre, AF.Sigmoid, bias=bf_c)
        nc.vector.tensor_copy(gate_b, gate_t)
        nc.vector.tensor_scalar(out=omg_b, in0=gate_t, scalar1=-1.0, scalar2=1.0,
                                op0=OP.mult, op1=OP.add)
        gd = dramp.tile([1, L], bf16, name="gd", tag="gd")
        od = dramp.tile([1, L], bf16, name="od", tag="od")
        nc.sync.dma_start(out=gd, in_=gate_b)
        nc.sync.dma_start(out=od, in_=omg_b)
        nc.sync.dma_start(out=gateomg[0:1, :], in_=bass_mod.AP(
            tensor=gd.tensor, offset=gd.offset, ap=[[0, 1], [1, NS], [NS, 128]]))
        nc.sync.dma_start(out=gateomg[1:2, :], in_=bass_mod.AP(
            tensor=od.tensor, offset=od.offset, ap=[[0, 1], [1, NS], [NS, 128]]))
        nc.sync.dma_start(out=bovg[0:1, :], in_=bo4)
        if _cut <= 2:
            return

        # Q/K projections -> qt/kt (bias evac split ACT/DVE)
        for c in range(2):
            for dst, w, bias in ((qt, wq, bq_t), (kt_, wk, bk_t)):
                for lo in range(0, L, 512):
                    ps = paq.tile([128, 512], f32, name="qkps", tag="qkps")
                    for k in range(KT):
                        nc.tensor.matmul(
                            out=ps, lhsT=w[k][:, c * 128:(c + 1) * 128],
                            rhs=xT[k][:, lo:lo + 512],
                            start=(k == 0), stop=(k == KT - 1))
                    if lo % 1024 == 0:
                        nc.scalar.activation(dst[c][:, lo:lo + 512], ps,
                                             AF.Identity, bias=bias[:, c:c + 1])
                    else:
                        nc.vector.tensor_scalar_add(
                            out=dst[c][:, lo:lo + 512], in0=ps,
                            scalar1=bias[:, c:c + 1])

        if _cut <= 3:
            return
        # global context: xsum on gpsimd, VG via N=1 matmuls
        import os as _os
        if _os.environ.get("K2_XSUM_DVE"):
            for k in range(KT):
                nc.vector.reduce_sum(out=xsum[:, k:k + 1], in_=xT[k], axis=X)
        else:
            xsum_scratch = consts.tile([128, L], bf16, name="xsum_scratch")
            for k in range(KT):
                nc.scalar.activation(xsum_scratch, xT[k], AF.Copy,
                                     accum_out=xsum[:, k:k + 1])
        nc.vector.tensor_copy(xsum_b, xsum)
        vgT = pag.tile([128, KT], f32, name="vgT", tag="vgT")
        for ch in range(KT):
            for k in range(KT):
                nc.tensor.matmul(out=vgT[:, ch:ch + 1],
                                 lhsT=wg[k][:, ch * 128:(ch + 1) * 128],
                                 rhs=xsum_b[:, k:k + 1],
                                 start=(k == 0), stop=(k == KT - 1))
        nc.vector.scalar_tensor_tensor(out=vg4T, in0=vgT, scalar=0.25 / L,
                                       op0=OP.mult, in1=bgT, op1=OP.add)
        vgd = dramp.tile([1, D], bf16, name="vgd", tag="vgd")
        nc.sync.dma_start(out=vgd, in_=vg4T)
        nc.sync.dma_start(out=bovg[1:2, :], in_=bass_mod.AP(
            tensor=vgd.tensor, offset=vgd.offset, ap=[[0, 1], [1, KT], [KT, 128]]))

    # ---- phase B/C: attention + output projection, chunk-major ----
    ex_sb = ctx.enter_context(tc.tile_pool(name="ex_sb", bufs=2))
    avn_sb = ctx.enter_context(tc.tile_pool(name="avn_sb", bufs=3))
    rc_sb = ctx.enter_context(tc.tile_pool(name="rc_sb", bufs=2))
    y_sb = ctx.enter_context(tc.tile_pool(name="y_sb", bufs=3))
    # single PSUM pool, per-tag bufs: st x2 + avs x4 + tr x1 + yps x1 = 8 banks
    psb = ctx.enter_context(tc.tile_pool(name="psb", bufs=1, space="PSUM"))
    stp = avp = trp = yp = psb

    ex = {}     # (h) -> list of 16 ex tiles for current chunk
    avs = {}    # qb -> psum tile [128, HG*(E+1)]
    yd = t["y"].rearrange("(t p) d -> t p d", p=128)

    def emit_scores(c, h):
        exl = []
        for s in range(NS):
            st = stp.tile([128, CHW], f32, name="st", tag="st", bufs=3)
            nc.tensor.matmul(
                out=st,
                lhsT=kt_[h // 2][64 * (h % 2):64 * (h % 2) + 64, s * 128:(s + 1) * 128],
                rhs=qt[h // 2][64 * (h % 2):64 * (h % 2) + 64, c * CHW:(c + 1) * CHW],
                start=True, stop=True)
            ext = ex_sb.tile([128, CHW], bf16, name="ex", tag=f"ex{s}")
            if s in DVE_KT_BY_H[h]:
                nc.vector.tensor_scalar(out=ext.bitcast(i16), in0=st,
                                        scalar1=SCH_A, scalar2=SCH_B,
                                        op0=OP.mult, op1=OP.add)
            else:
                nc.scalar.activation(ext, st, AF.Exp, scale=SCALE)
            exl.append(ext)
        ex[h] = exl

    def emit_av(c, h):
        if h == 0:
            avs[0] = avp.tile([128, 2 * CW], f32, name="avsA", tag="avsA")
            avs[1] = avp.tile([128, 2 * CW], f32, name="avsB", tag="avsB")
            avs["d"] = avp.tile([128, HG * NQB], f32, name="avsd", tag="avsd")
        for qb in range(NQB):
            lo = (qb % 2) * CW + h * E
            dcol = h * NQB + qb
            for s in range(NS):
                exsl = ex[h][s][:, qb * 128:(qb + 1) * 128]
                nc.tensor.matmul(
                    out=avs[qb // 2][:, lo:lo + E],
                    lhsT=exsl, rhs=v_aug[s][:, h * E:(h + 1) * E],
                    start=(s == 0), stop=(s == NS - 1))
                nc.tensor.matmul(
                    out=avs["d"][:, dcol:dcol + 1],
                    lhsT=exsl, rhs=ones_c,
                    start=(s == 0), stop=(s == NS - 1))

    def emit_norm(c):
        # normalize + transpose into vt for chunk c
        for qb in range(NQB):
            lt = c * NQB + qb
            d4 = avs["d"].rearrange("p (h q) -> p h q", q=NQB)[:, :, qb]
            rc4 = rc_sb.tile([128, HG], f32, name="rc4", tag="rc4")
            nc.vector.reciprocal(rc4, d4)
            nc.vector.tensor_scalar(out=rc4, in0=rc4,
                                    scalar1=gate_t[:, lt:lt + 1], scalar2=None,
                                    op0=OP.mult)
            avn = avn_sb.tile([128, HG * E], bf16, name="avn", tag="avn")
            rc_b = bass_mod.AP(tensor=rc4.tensor, offset=rc4.offset,
                               ap=[[rc4.ap[0][0], 128], [1, HG], [0, E]])
            src_ap = avs[qb // 2][:, (qb % 2) * CW:(qb % 2) * CW + CW]
            nc.vector.tensor_tensor(
                out=avn.rearrange("p (h c) -> p h c", c=E),
                in0=src_ap.rearrange("p (h c) -> p h c", c=E),
                in1=rc_b, op=OP.mult)
            for kc in range(2):
                tr = trp.tile([128, 128], bf16, name="tr", tag="tail", bufs=2)
                nc.tensor.transpose(tr, avn[:, kc * 128:(kc + 1) * 128], ident)
                nc.vector.tensor_copy(vt[kc][:, lt * 128:(lt + 1) * 128], tr)

    def emit_wo(c):
        # output projection + bias fusion + evac + DMA for chunk c
        for qb in range(NQB):
            lt = c * NQB + qb
            lsl = slice(lt * 128, (lt + 1) * 128)
            for do in range(0, D, 512):
                ps = yp.tile([128, 512], f32, name="yps", tag="tail", bufs=2)
                nc.tensor.matmul(out=ps, lhsT=vt[0][:, lsl],
                                 rhs=wo[0][:, do:do + 512], start=True, stop=False)
                nc.tensor.matmul(out=ps, lhsT=vt[1][:, lsl],
                                 rhs=wo[1][:, do:do + 512], start=False, stop=False)
                nc.tensor.matmul(out=ps, lhsT=gateomg[:, lsl],
                                 rhs=bovg[:, do:do + 512], start=False, stop=True)
                ot = y_sb.tile([128, 512], f32, name="ot", tag="ot")
                nc.scalar.activation(ot, ps, AF.Copy)
                nc.sync.dma_start(out=yd[lt][:, do:do + 512], in_=ot)

    # interleaved emission: scores run ahead of AV; each chunk's norm and
    # output projection slot between the next chunk's first score blocks so
    # no engine head-of-line blocks.
    import os
    _stop = os.environ.get("K2_STOP", "")
    if _stop == "A":
        return
    for c in range(NCH):
        if c == 0:
            emit_scores(0, 0)
            emit_scores(0, 1)
        emit_av(c, 0)
        emit_scores(c, 2)
        emit_av(c, 1)
        emit_scores(c, 3)
        emit_av(c, 2)
        emit_av(c, 3)
        if c + 1 < NCH:
            emit_scores(c + 1, 0)
            emit_norm(c)
            emit_scores(c + 1, 1)
            emit_wo(c)
        else:
            emit_norm(c)
            emit_wo(c)
        if _stop == "1":
            return


def _emit_main(nc, tile, mybir, ctx, tc, t):
    _emit(nc, tile, mybir, ctx, tc, t)


def _build():
    if "nc" in _CACHED:
        return _CACHED["nc"]
    import concourse.bass as bass
    import concourse.tile as tile
    from concourse import mybir
    from contextlib import ExitStack

    _patch_drain(tile, mybir)
    nc = bass.Bass("TRN2", target_bir_lowering=False, debug=False)
    f32, bf16 = mybir.dt.float32, mybir.dt.bfloat16
    t = {
        "xT": nc.dram_tensor("xT", [128, KT * L], bf16, kind="ExternalInput").ap(),
        "cb": nc.dram_tensor("cb", [128, 5 + CW], f32, kind="ExternalInput").ap(),
        "wq": nc.dram_tensor("wq", [128, KT * CW], bf16, kind="ExternalInput").ap(),
        "wk": nc.dram_tensor("wk", [128, KT * CW], bf16, kind="ExternalInput").ap(),
        "wv": nc.dram_tensor("wv", [128, KT * CW], bf16, kind="ExternalInput").ap(),
        "wfb": nc.dram_tensor("wfb", [128, KT], bf16, kind="ExternalInput").ap(),
        "wo": nc.dram_tensor("wo", [128, 2 * D], bf16, kind="ExternalInput").ap(),
        "wg": nc.dram_tensor("wg", [128, KT * D], bf16, kind="ExternalInput").ap(),
        "ident": nc.dram_tensor("ident", [128, 128], bf16, kind="ExternalInput").ap(),
        "bo4": nc.dram_tensor("bo4", [1, D], bf16, kind="ExternalInput").ap(),
        "bgT": nc.dram_tensor("bgT", [128, KT], f32, kind="ExternalInput").ap(),
        "y": nc.dram_tensor("y", [L, D], f32, kind="ExternalOutput").ap(),
    }
    with tile.TileContext(nc) as tc:
        with ExitStack() as ctx:
            _emit_main(nc, tile, mybir, ctx, tc, t)
    if SPLIT_WAITS:
        _split_multi_waits(nc, mybir)
    _CACHED["nc"] = nc
    return nc


def _pack(w):
    """[KT*128, C] -> [128, KT*C] with k-tiles side by side."""
    kt = w.shape[0] // 128
    return np.ascontiguousarray(
        w.reshape(kt, 128, w.shape[1]).transpose(1, 0, 2).reshape(128, -1))


def _prep_core_inputs(c, inputs, shared):
    b, g = c // 4, c % 4
    cols = slice(g * CW, (g + 1) * CW)
    bf_val = float(np.asarray(inputs["bf"]).reshape(-1)[0])
    m = {
        "xT": shared["xT"][b],
        "wq": _pack(inputs["Wq"][:, cols]).astype(BF16),
        "wk": _pack(inputs["Wk"][:, cols]).astype(BF16),
        "wv": _pack(inputs["Wv"][:, cols]).astype(BF16),
        "wfb": shared["wfb"],
        "wo": _pack(inputs["Wo"][cols, :]).astype(BF16),
        "wg": shared["wg"],
        "ident": shared["ident"],
        "cb": np.concatenate([
            inputs["bq"][cols].reshape(2, 128).T,
            inputs["bk"][cols].reshape(2, 128).T,
            np.full((128, 1), bf_val, np.float32),
            np.broadcast_to(inputs["bv"][cols][None, :], (128, CW)),
        ], axis=1).astype(np.float32),
        "bo4": (inputs["bo"][None, :] * 0.25).astype(BF16),
        "bgT": (inputs["bg"].reshape(KT, 128).T * 0.25).astype(np.float32),
    }
    return m


def kernel(**inputs):
    from concourse import bass_utils

    nc = _build()
    shared = {
        "xT": [_pack(inputs["x"][b].T).astype(BF16) for b in range(B)],
        "wg": _pack(inputs["Wg"]).astype(BF16),
        "wfb": np.ascontiguousarray(inputs["Wf"].reshape(KT, 128).T).astype(BF16),
        "ident": np.eye(128, dtype=np.float32).astype(BF16),
    }
    in_maps = [_prep_core_inputs(c, inputs, shared) for c in range(N_CORES)]
    res = bass_utils.run_bass_kernel_spmd(nc, in_maps, core_ids=list(range(N_CORES)))
    out = np.zeros((B, L, D), np.float32)
    for c in range(N_CORES):
        out[c // 4] += res.results[c]["y"]
    return out


# revision 4
# speedup vs baseline: 1.0901x; 1.0439x over previous
"""Trainium2 Bass kernel v2 for the gated-attention layer.

Sharding: 8 cores = (2 batches) x (4 head-groups of 4 heads each), as v1.

Key structure changes vs v1:
- Flipped AV matmul: out[q, head_dim] = ex_tile.T @ v_slice with N=65 per
  matmul (cost-model charges by output free size) -> AV cost halves, and
  denominators land per-partition (no DMA-bounce transposes).
- VG and gate pre-activations via N=1 matmuls (out [128,1] per k-tile).
- exp split between ScalarE (exact) and DVE (Schraudolph int16->bf16
  bitcast) so ACT is not the attention bottleneck.
- chunk-major loop (512 queries) with interleaved emission so PE never
  head-of-line blocks on exp.
- Output projection per chunk; y evacuated on ACT, DMA'd per 512-col tile.
"""

import sys

for _p in ("/root/.axon_site/_ro/trn_rl_repo", "/opt/trn_rl_repo"):
    if _p not in sys.path:
        sys.path.append(_p)

import numpy as np
import ml_dtypes

B, L, D, H = 2, 2048, 1024, 16
E = D // H          # 64 head dim
N_CORES = 8
HG = 4              # heads per core
CW = HG * E         # 256 cols per core
KT = D // 128       # 8 contraction k-tiles
NS = L // 128       # 16 key tiles
NCH = 4             # query chunks
CHW = L // NCH      # 512 queries per chunk
NQB = CHW // 128    # 4 q-blocks per chunk
SCALE = 1.0 / np.sqrt(E)

# Schraudolph exp on DVE for these key-tiles (rest on ACT); heads 0/1 are
# scored inside the tail window where DVE also runs norm/transpose-evac,
# so they get fewer DVE tiles.
DVE_KT_BY_H = {
    0: (1, 3, 5, 7, 9, 11, 13),
    1: (2, 4, 6, 8, 10, 12, 14),
    2: (0, 2, 4, 6, 8, 10, 12, 14, 15),
    3: (1, 3, 5, 7, 9, 11, 13, 15, 0),
}
SCH_A = 128.0 / np.log(2.0) * SCALE    # 23.083
WSCALE = 32.0   # qkv weights scaled up for fp8 hi-lo (denormal floor), undone at evac
SCH_B = 16250.5                        # 127*128 - 5.5 (centering) + 0.5 (trunc)

BF16 = ml_dtypes.bfloat16

_CACHED = {}
SPLIT_WAITS = True  # walrus HW build needs single-wait insts; CoreSim chokes on the split NoOps


def _patch_drain(tile_mod, mybir):
    """This walrus build only accepts one sync-wait on a Drain; spread the
    final Tile drain's waits over single-wait NOPs."""
    from concourse.vector_clock import ScopedClock

    def _dab(self, tick_clock, wait_clock):
        nc = self.nc
        drain_inst = nc.sync.drain()
        wait_clock.add_sem_waits(
            drain_inst.ins, ScopedClock({None: tick_clock.global_clock})
        )
        waits = list(drain_inst.ins.sync_info.on_wait)
        if len(waits) > 1:
            drain_inst.ins.sync_info.on_wait = waits[:1]
            for w in waits[1:]:
                nop = nc.sync.nop()
                if nop.ins.sync_info is None:
                    nop.ins.sync_info = mybir.SyncInfo(on_wait=[w], on_update=[])
                else:
                    nop.ins.sync_info.on_wait = [w]
        nc.all_engine_barrier()
        assert self.sems is not None
        popped = nc._tile_sem_poison_stack.pop()
        assert popped is self._sem_poison
        nc.clear_and_free_semaphores(list(self.sems.allocated().values()))
        nc.all_engine_barrier()

    tile_mod.TileContext._drain_and_barrier = _dab


def _split_multi_waits(nc, mybir):
    """One sync-wait per instruction; move extras onto same-engine NOPs."""
    ctr = 0
    for blk in nc.m.functions[0].blocks:
        insts = list(blk.instructions)
        out = []
        for inst in insts:
            si = getattr(inst, "sync_info", None)
            if si is not None and si.on_wait is not None and len(si.on_wait) > 1:
                waits = list(si.on_wait)
                for w in waits[:-1]:
                    nop = mybir.InstNoOp(
                        name=f"I-waitsplit-{ctr}",
                        engine=inst.engine,
                        sync_info=mybir.SyncInfo(on_wait=[w], on_update=[]),
                        bass_nofuse=True,
                    )
                    ctr += 1
                    out.append(nop)
                si.on_wait = waits[-1:]
            out.append(inst)
        if len(out) != len(insts):
            blk.instructions[:] = out


def _emit(nc, tile, mybir, ctx, tc, t):
    import concourse.bass as bass_mod

    f32 = mybir.dt.float32
    bf16 = mybir.dt.bfloat16
    i16 = mybir.dt.int16
    AF = mybir.ActivationFunctionType
    OP = mybir.AluOpType
    X = mybir.AxisListType.X

    consts = ctx.enter_context(tc.tile_pool(name="consts", bufs=1))
    dramp = ctx.enter_context(tc.tile_pool(name="dramp", bufs=2, space="DRAM"))

    # ---- SBUF constants / persistent tiles ----
    cb = consts.tile([128, 5 + CW], f32)
    nc.sync.dma_start(out=cb, in_=t["cb"])
    bq_t, bk_t = cb[:, 0:2], cb[:, 2:4]
    bf_c = cb[:, 4:5]
    bv_b = cb[:, 5:5 + CW]

    # all host-side tensors are packed [128, KT*cols] (k-tiles side by side);
    # x and qkv weights come as fp8 hi+lo pairs for DoubleRow hi-lo matmuls
    f8 = mybir.dt.float8e4
    xh_all = consts.tile([128, KT * L], f8, name="xh_all", tag="xh_all")
    xl_all = consts.tile([128, KT * L], f8, name="xl_all", tag="xl_all")
    xh = [xh_all[:, k * L:(k + 1) * L] for k in range(KT)]
    xl = [xl_all[:, k * L:(k + 1) * L] for k in range(KT)]

    def w_packed(name, cols, dt=bf16):
        all_t = consts.tile([128, KT * cols], dt, name=f"{name}_all",
                            tag=f"{name}_all")
        return all_t, [all_t[:, k * cols:(k + 1) * cols] for k in range(KT)]

    wqh_all, _ = w_packed("wqh", CW, f8)
    wql_all, _ = w_packed("wql", CW, f8)
    wkh_all, _ = w_packed("wkh", CW, f8)
    wkl_all, _ = w_packed("wkl", CW, f8)
    wvh_all, _ = w_packed("wvh", CW, f8)
    wvl_all, _ = w_packed("wvl", CW, f8)
    wg_all, wg = w_packed("wg", D)

    def dr3(big, off, mid, n):
        # [128, 2, n] DoubleRow AP into packed tile `big` at element offset
        return bass_mod.AP(tensor=big.tensor, offset=big.offset + off,
                           ap=[[big.ap[0][0], 128], [mid, 2], [1, n]])
    wfb = consts.tile([128, KT], bf16)
    ident = consts.tile([128, 128], bf16)
    bo4 = consts.tile([1, D], mybir.dt.float8e4)
    bgT = consts.tile([128, KT], f32)
    wo_all = consts.tile([128, 2 * D], bf16, name="wo_all", tag="wo_all")
    wo = [wo_all[:, k * D:(k + 1) * D] for k in range(2)]

    # few, large DMAs (HWDGE issue is a serialized ~630ns/DMA resource):
    # wv + wfb first, x hi/lo in 2-ktile pieces, then wq/wk, wo, wg.
    nc.sync.dma_start(out=wvh_all, in_=t["wvh"])
    nc.scalar.dma_start(out=wvl_all, in_=t["wvl"])
    nc.scalar.dma_start(out=wfb, in_=t["wfb"])
    for kp in range(KT // 2):
        nc.sync.dma_start(out=xh_all[:, 2 * kp * L:(2 * kp + 2) * L],
                          in_=t["xh"][:, 2 * kp * L:(2 * kp + 2) * L])
        nc.scalar.dma_start(out=xl_all[:, 2 * kp * L:(2 * kp + 2) * L],
                            in_=t["xl"][:, 2 * kp * L:(2 * kp + 2) * L])
    nc.scalar.dma_start(out=wqh_all, in_=t["wqh"])
    nc.scalar.dma_start(out=wql_all, in_=t["wql"])
    nc.scalar.dma_start(out=wkh_all, in_=t["wkh"])
    nc.scalar.dma_start(out=wkl_all, in_=t["wkl"])
    nc.sync.dma_start(out=wo_all, in_=t["wo"])
    nc.scalar.dma_start(out=ident, in_=t["ident"])
    nc.scalar.dma_start(out=bo4, in_=t["bo4"])
    nc.scalar.dma_start(out=bgT, in_=t["bgT"])
    for half in range(2):
        nc.sync.dma_start(out=wg_all[:, half * 4 * D:(half + 1) * 4 * D],
                          in_=t["wg"][:, half * 4 * D:(half + 1) * 4 * D])

    qt = [consts.tile([128, L], bf16, name=f"qt{i}", tag=f"qt{i}") for i in range(2)]
    kt_ = [consts.tile([128, L], bf16, name=f"kt{i}", tag=f"kt{i}") for i in range(2)]
    v_aug = [consts.tile([128, CW], bf16, name=f"va{i}", tag=f"va{i}")
             for i in range(NS)]
    ones_c = consts.tile([128, 1], bf16)
    vt = [consts.tile([128, L], bf16, name=f"vt{i}", tag=f"vt{i}") for i in range(2)]
    gate_t = consts.tile([128, NS], f32)
    gate_b = consts.tile([128, NS], mybir.dt.float8e4)
    omg_b = consts.tile([128, NS], mybir.dt.float8e4)
    gateomg = consts.tile([1, 2 * L], mybir.dt.float8e4)
    bovg = consts.tile([1, 2 * D], mybir.dt.float8e4)
    xsum = consts.tile([128, KT], f32)
    xsum_b = consts.tile([128, KT], bf16)
    vg4T = consts.tile([128, KT], mybir.dt.float8e4)

    nc.vector.memset(ones_c, 1.0)

    # ---- phase A: projections, gate, global context ----
    import os as _os0
    _cut = int(_os0.environ.get("K2_CUT", "99"))
    if _cut == 0:
        return
    with tc.tile_pool(name="pav", bufs=2, space="PSUM") as pav, \
         tc.tile_pool(name="paq", bufs=2, space="PSUM") as paq, \
         tc.tile_pool(name="pag", bufs=1, space="PSUM") as pag:
        # V projection + per-tile evac with bias
        _nv = int(_os0.environ.get("K2_NV", str(NS)))
        _noevac = _os0.environ.get("K2_NOEVAC")
        NKP = KT // 2
        DR = mybir.MatmulPerfMode.DoubleRow
        for s in range(_nv):
            ps = pav.tile([128, CW], f32, name="vps", tag="vps")
            terms = ((xh_all, wvh_all), (xh_all, wvl_all), (xl_all, wvh_all))
            for ti, (xa, wa) in enumerate(terms):
                for kp in range(NKP):
                    nc.tensor.matmul(
                        out=ps,
                        lhsT=dr3(xa, 2 * kp * L + s * 128, L, 128),
                        rhs=dr3(wa, 2 * kp * CW, CW, CW),
                        start=(ti == 0 and kp == 0),
                        stop=(ti == 2 and kp == NKP - 1), perf_mode=DR)
            if not _noevac:
                nc.vector.scalar_tensor_tensor(out=v_aug[s], in0=ps,
                                               scalar=1.0 / WSCALE, op0=OP.mult,
                                               in1=bv_b, op1=OP.add)
        if _cut <= 1:
            return
        # gate pre-activations via N=1 matmuls, one sigmoid
        gpre = pag.tile([128, NS], f32, name="gpre", tag="gpre")
        for s in range(NS):
            for xi, xa in enumerate((xh_all, xl_all)):
                for k in range(KT):
                    nc.tensor.matmul(
                        out=gpre[:, s:s + 1],
                        lhsT=bass_mod.AP(tensor=xa.tensor,
                                         offset=xa.offset + k * L + s * 128,
                                         ap=[[xa.ap[0][0], 128], [1, 128]]),
                        rhs=wfb[:, k:k + 1],
                        start=(xi == 0 and k == 0),
                        stop=(xi == 1 and k == KT - 1))
        nc.scalar.activation(gate_t, gpre, AF.Sigmoid, bias=bf_c)
        nc.vector.tensor_copy(gate_b, gate_t)
        nc.vector.tensor_scalar(out=omg_b, in0=gate_t, scalar1=-1.0, scalar2=1.0,
                                op0=OP.mult, op1=OP.add)
        gd = dramp.tile([1, L], mybir.dt.float8e4, name="gd", tag="gd")
        od = dramp.tile([1, L], mybir.dt.float8e4, name="od", tag="od")
        nc.sync.dma_start(out=gd, in_=gate_b)
        nc.sync.dma_start(out=od, in_=omg_b)
        nc.sync.dma_start(out=gateomg[:, 0:L], in_=bass_mod.AP(
            tensor=gd.tensor, offset=gd.offset, ap=[[0, 1], [1, NS], [NS, 128]]))
        nc.sync.dma_start(out=gateomg[:, L:2 * L], in_=bass_mod.AP(
            tensor=od.tensor, offset=od.offset, ap=[[0, 1], [1, NS], [NS, 128]]))
        nc.sync.dma_start(out=bovg[:, 0:D], in_=bo4)
        if _cut <= 2:
            return

        # Q/K projections -> qt/kt (bias evac split ACT/DVE)
        for c in range(2):
            for dst, wh, wl, bias in ((qt, wqh_all, wql_all, bq_t),
                                      (kt_, wkh_all, wkl_all, bk_t)):
                for lo in range(0, L, 512):
                    ps = paq.tile([128, 512], f32, name="qkps", tag="qkps")
                    terms = ((wh, xh_all), (wl, xh_all), (wh, xl_all))
                    for ti, (wa, xa) in enumerate(terms):
                        for kp in range(NKP):
                            nc.tensor.matmul(
                                out=ps,
                                lhsT=dr3(wa, 2 * kp * CW + c * 128, CW, 128),
                                rhs=dr3(xa, 2 * kp * L + lo, L, 512),
                                start=(ti == 0 and kp == 0),
                                stop=(ti == 2 and kp == NKP - 1), perf_mode=DR)
                    if lo % 1024 == 0:
                        nc.scalar.activation(dst[c][:, lo:lo + 512], ps,
                                             AF.Identity, bias=bias[:, c:c + 1],
                                             scale=1.0 / WSCALE)
                    else:
                        nc.vector.tensor_scalar(
                            out=dst[c][:, lo:lo + 512], in0=ps,
                            scalar1=1.0 / WSCALE, scalar2=bias[:, c:c + 1],
                            op0=OP.mult, op1=OP.add)

        if _cut <= 3:
            return
        # global context: xsum on gpsimd, VG via N=1 matmuls
        xsum_scratch = consts.tile([128, L], bf16, name="xsum_scratch")
        xsuml = consts.tile([128, KT], f32, name="xsuml")
        for k in range(KT):
            nc.scalar.activation(xsum_scratch, xh[k], AF.Copy,
                                 accum_out=xsum[:, k:k + 1])
            nc.vector.reduce_sum(out=xsuml[:, k:k + 1], in_=xl[k], axis=X)
        nc.vector.tensor_add(xsum, xsum, xsuml)
        nc.vector.tensor_copy(xsum_b, xsum)
        vgT = pag.tile([128, KT], f32, name="vgT", tag="vgT")
        for ch in range(KT):
            for k in range(KT):
                nc.tensor.matmul(out=vgT[:, ch:ch + 1],
                                 lhsT=wg[k][:, ch * 128:(ch + 1) * 128],
                                 rhs=xsum_b[:, k:k + 1],
                                 start=(k == 0), stop=(k == KT - 1))
        nc.vector.scalar_tensor_tensor(out=vg4T, in0=vgT, scalar=0.25 / L,
                                       op0=OP.mult, in1=bgT, op1=OP.add)
        vgd = dramp.tile([1, D], mybir.dt.float8e4, name="vgd", tag="vgd")
        nc.sync.dma_start(out=vgd, in_=vg4T)
        nc.sync.dma_start(out=bovg[:, D:2 * D], in_=bass_mod.AP(
            tensor=vgd.tensor, offset=vgd.offset, ap=[[0, 1], [1, KT], [KT, 128]]))

    # ---- phase B/C: attention + output projection, chunk-major ----
    ex_sb = ctx.enter_context(tc.tile_pool(name="ex_sb", bufs=2))
    avn_sb = ctx.enter_context(tc.tile_pool(name="avn_sb", bufs=3))
    rc_sb = ctx.enter_context(tc.tile_pool(name="rc_sb", bufs=2))
    y_sb = ctx.enter_context(tc.tile_pool(name="y_sb", bufs=3))
    # single PSUM pool, per-tag bufs: st x2 + avs x4 + tr x1 + yps x1 = 8 banks
    psb = ctx.enter_context(tc.tile_pool(name="psb", bufs=1, space="PSUM"))
    stp = avp = trp = yp = psb

    ex = {}     # (h) -> list of 16 ex tiles for current chunk
    avs = {}    # qb -> psum tile [128, HG*(E+1)]
    yd = t["y"].rearrange("(t p) d -> t p d", p=128)

    def emit_scores(c, h):
        exl = []
        for s in range(NS):
            st = stp.tile([128, CHW], f32, name="st", tag="st", bufs=3)
            nc.tensor.matmul(
                out=st,
                lhsT=kt_[h // 2][64 * (h % 2):64 * (h % 2) + 64, s * 128:(s + 1) * 128],
                rhs=qt[h // 2][64 * (h % 2):64 * (h % 2) + 64, c * CHW:(c + 1) * CHW],
                start=True, stop=True)
            ext = ex_sb.tile([128, CHW], bf16, name="ex", tag=f"ex{s}", bufs=3)
            if s in DVE_KT_BY_H[h]:
                nc.vector.tensor_scalar(out=ext.bitcast(i16), in0=st,
                                        scalar1=SCH_A, scalar2=SCH_B,
                                        op0=OP.mult, op1=OP.add)
            else:
                nc.scalar.activation(ext, st, AF.Exp, scale=SCALE)
            exl.append(ext)
        ex[h] = exl

    def emit_av(c, h):
        if h == 0:
            avs[0] = avp.tile([128, 2 * CW], f32, name="avsA", tag="avsA")
            avs[1] = avp.tile([128, 2 * CW], f32, name="avsB", tag="avsB")
            avs["d"] = avp.tile([128, HG * NQB], f32, name="avsd", tag="avsd")
        for qb in range(NQB):
            lo = (qb % 2) * CW + h * E
            dcol = h * NQB + qb
            for s in range(NS):
                exsl = ex[h][s][:, qb * 128:(qb + 1) * 128]
                nc.tensor.matmul(
                    out=avs[qb // 2][:, lo:lo + E],
                    lhsT=exsl, rhs=v_aug[s][:, h * E:(h + 1) * E],
                    start=(s == 0), stop=(s == NS - 1))
                nc.tensor.matmul(
                    out=avs["d"][:, dcol:dcol + 1],
                    lhsT=exsl, rhs=ones_c,
                    start=(s == 0), stop=(s == NS - 1))

    def emit_norm(c):
        # normalize + transpose into vt for chunk c
        for qb in range(NQB):
            lt = c * NQB + qb
            d4 = avs["d"].rearrange("p (h q) -> p h q", q=NQB)[:, :, qb]
            rc4 = rc_sb.tile([128, HG], f32, name="rc4", tag="rc4")
            nc.vector.reciprocal(rc4, d4)
            nc.vector.tensor_scalar(out=rc4, in0=rc4,
                                    scalar1=gate_t[:, lt:lt + 1], scalar2=None,
                                    op0=OP.mult)
            avn = avn_sb.tile([128, HG * E], bf16, name="avn", tag="avn")
            rc_b = bass_mod.AP(tensor=rc4.tensor, offset=rc4.offset,
                               ap=[[rc4.ap[0][0], 128], [1, HG], [0, E]])
            src_ap = avs[qb // 2][:, (qb % 2) * CW:(qb % 2) * CW + CW]
            nc.vector.tensor_tensor(
                out=avn.rearrange("p (h c) -> p h c", c=E),
                in0=src_ap.rearrange("p (h c) -> p h c", c=E),
                in1=rc_b, op=OP.mult)
            for kc in range(2):
                tr = trp.tile([128, 128], bf16, name="tr", tag="tail", bufs=2)
                nc.tensor.transpose(tr, avn[:, kc * 128:(kc + 1) * 128], ident)
                nc.scalar.activation(vt[kc][:, lt * 128:(lt + 1) * 128], tr,
                                     AF.Copy)

    def emit_wo(c):
        # output projection + bias fusion + evac + DMA for chunk c
        for qb in range(NQB):
            lt = c * NQB + qb
            lsl = slice(lt * 128, (lt + 1) * 128)
            for do in range(0, D, 512):
                ps = yp.tile([128, 512], f32, name="yps", tag="tail", bufs=2)
                nc.tensor.matmul(out=ps, lhsT=vt[0][:, lsl],
                                 rhs=wo[0][:, do:do + 512], start=True, stop=False)
                nc.tensor.matmul(out=ps, lhsT=vt[1][:, lsl],
                                 rhs=wo[1][:, do:do + 512], start=False, stop=False)
                gdr = bass_mod.AP(tensor=gateomg.tensor, offset=gateomg.offset
                                  + lt * 128, ap=[[gateomg.ap[0][0], 1], [L, 2],
                                                  [1, 128]])
                bdr = bass_mod.AP(tensor=bovg.tensor, offset=bovg.offset + do,
                                  ap=[[bovg.ap[0][0], 1], [D, 2], [1, 512]])
                nc.tensor.matmul(out=ps, lhsT=gdr, rhs=bdr, start=False,
                                 stop=True, perf_mode=mybir.MatmulPerfMode.DoubleRow)
                ot = y_sb.tile([128, 512], bf16, name="ot", tag="ot")
                if do == 0:
                    nc.scalar.activation(ot, ps, AF.Copy)
                else:
                    nc.vector.tensor_copy(ot, ps)
                nc.sync.dma_start(out=yd[lt][:, do:do + 512], in_=ot)

    # interleaved emission: scores run ahead of AV; each chunk's norm and
    # output projection slot between the next chunk's first score blocks so
    # no engine head-of-line blocks.
    import os
    _stop = os.environ.get("K2_STOP", "")
    if _stop == "A":
        return
    for c in range(NCH):
        if c == 0:
            emit_scores(0, 0)
            emit_scores(0, 1)
        emit_av(c, 0)
        emit_scores(c, 2)
        emit_av(c, 1)
        emit_scores(c, 3)
        emit_av(c, 2)
        if c + 1 < NCH:
            emit_scores(c + 1, 0)
        emit_av(c, 3)
        if c + 1 < NCH:
            emit_scores(c + 1, 1)
        emit_norm(c)
        emit_wo(c)
        if _stop == "1":
            return


def _emit_main(nc, tile, mybir, ctx, tc, t):
    _emit(nc, tile, mybir, ctx, tc, t)


def _build():
    if "nc" in _CACHED:
        return _CACHED["nc"]
    import concourse.bass as bass
    import concourse.tile as tile
    from concourse import mybir
    from contextlib import ExitStack

    _patch_drain(tile, mybir)
    nc = bass.Bass("TRN2", target_bir_lowering=False, debug=False)
    f32, bf16 = mybir.dt.float32, mybir.dt.bfloat16
    t = {
        "xh": nc.dram_tensor("xh", [128, KT * L], mybir.dt.float8e4, kind="ExternalInput").ap(),
        "xl": nc.dram_tensor("xl", [128, KT * L], mybir.dt.float8e4, kind="ExternalInput").ap(),
        "cb": nc.dram_tensor("cb", [128, 5 + CW], f32, kind="ExternalInput").ap(),
        "wqh": nc.dram_tensor("wqh", [128, KT * CW], mybir.dt.float8e4, kind="ExternalInput").ap(),
        "wql": nc.dram_tensor("wql", [128, KT * CW], mybir.dt.float8e4, kind="ExternalInput").ap(),
        "wkh": nc.dram_tensor("wkh", [128, KT * CW], mybir.dt.float8e4, kind="ExternalInput").ap(),
        "wkl": nc.dram_tensor("wkl", [128, KT * CW], mybir.dt.float8e4, kind="ExternalInput").ap(),
        "wvh": nc.dram_tensor("wvh", [128, KT * CW], mybir.dt.float8e4, kind="ExternalInput").ap(),
        "wvl": nc.dram_tensor("wvl", [128, KT * CW], mybir.dt.float8e4, kind="ExternalInput").ap(),
        "wfb": nc.dram_tensor("wfb", [128, KT], bf16, kind="ExternalInput").ap(),
        "wo": nc.dram_tensor("wo", [128, 2 * D], bf16, kind="ExternalInput").ap(),
        "wg": nc.dram_tensor("wg", [128, KT * D], bf16, kind="ExternalInput").ap(),
        "ident": nc.dram_tensor("ident", [128, 128], bf16, kind="ExternalInput").ap(),
        "bo4": nc.dram_tensor("bo4", [1, D], mybir.dt.float8e4, kind="ExternalInput").ap(),
        "bgT": nc.dram_tensor("bgT", [128, KT], f32, kind="ExternalInput").ap(),
        "y": nc.dram_tensor("y", [L, D], bf16, kind="ExternalOutput").ap(),
    }
    with tile.TileContext(nc) as tc:
        with ExitStack() as ctx:
            _emit_main(nc, tile, mybir, ctx, tc, t)
    if SPLIT_WAITS:
        _split_multi_waits(nc, mybir)
    _CACHED["nc"] = nc
    return nc


def _pack(w):
    """[KT*128, C] -> [128, KT*C] with k-tiles side by side."""
    kt = w.shape[0] // 128
    return np.ascontiguousarray(
        w.reshape(kt, 128, w.shape[1]).transpose(1, 0, 2).reshape(128, -1))


F8 = ml_dtypes.float8_e4m3fn


def _hilo(a):
    hi = a.astype(F8)
    lo = (a - hi.astype(np.float32)).astype(F8)
    return hi, lo


def _prep_core_inputs(c, inputs, shared):
    b, g = c // 4, c % 4
    cols = slice(g * CW, (g + 1) * CW)
    bf_val = float(np.asarray(inputs["bf"]).reshape(-1)[0])
    wqh, wql = _hilo(_pack(inputs["Wq"][:, cols]) * WSCALE)
    wkh, wkl = _hilo(_pack(inputs["Wk"][:, cols]) * WSCALE)
    wvh, wvl = _hilo(_pack(inputs["Wv"][:, cols]) * WSCALE)
    m = {
        "xh": shared["xh"][b], "xl": shared["xl"][b],
        "wqh": wqh, "wql": wql, "wkh": wkh, "wkl": wkl,
        "wvh": wvh, "wvl": wvl,
        "wfb": shared["wfb"],
        "wo": _pack(inputs["Wo"][cols, :]).astype(BF16),
        "wg": shared["wg"],
        "ident": shared["ident"],
        "cb": np.concatenate([
            inputs["bq"][cols].reshape(2, 128).T,
            inputs["bk"][cols].reshape(2, 128).T,
            np.full((128, 1), bf_val, np.float32),
            np.broadcast_to(inputs["bv"][cols][None, :], (128, CW)),
        ], axis=1).astype(np.float32),
        "bo4": (inputs["bo"][None, :] * 0.25).astype(ml_dtypes.float8_e4m3fn),
        "bgT": (inputs["bg"].reshape(KT, 128).T * 0.25).astype(np.float32),
    }
    return m


def kernel(**inputs):
    from concourse import bass_utils

    nc = _build()
    xhl = [_hilo(_pack(inputs["x"][b].T)) for b in range(B)]
    shared = {
        "xh": [xhl[b][0] for b in range(B)],
        "xl": [xhl[b][1] for b in range(B)],
        "wg": _pack(inputs["Wg"]).astype(BF16),
        "wfb": np.ascontiguousarray(inputs["Wf"].reshape(KT, 128).T).astype(BF16),
        "ident": np.eye(128, dtype=np.float32).astype(BF16),
    }
    in_maps = [_prep_core_inputs(c, inputs, shared) for c in range(N_CORES)]
    res = bass_utils.run_bass_kernel_spmd(nc, in_maps, core_ids=list(range(N_CORES)))
    out = np.zeros((B, L, D), np.float32)
    for c in range(N_CORES):
        out[c // 4] += res.results[c]["y"]
    return out


# revision 5
# speedup vs baseline: 1.1017x; 1.0107x over previous
"""Trainium2 Bass kernel v2 for the gated-attention layer.

Sharding: 8 cores = (2 batches) x (4 head-groups of 4 heads each), as v1.

Key structure changes vs v1:
- Flipped AV matmul: out[q, head_dim] = ex_tile.T @ v_slice with N=65 per
  matmul (cost-model charges by output free size) -> AV cost halves, and
  denominators land per-partition (no DMA-bounce transposes).
- VG and gate pre-activations via N=1 matmuls (out [128,1] per k-tile).
- exp split between ScalarE (exact) and DVE (Schraudolph int16->bf16
  bitcast) so ACT is not the attention bottleneck.
- chunk-major loop (512 queries) with interleaved emission so PE never
  head-of-line blocks on exp.
- Output projection per chunk; y evacuated on ACT, DMA'd per 512-col tile.
"""

import sys

for _p in ("/root/.axon_site/_ro/trn_rl_repo", "/opt/trn_rl_repo"):
    if _p not in sys.path:
        sys.path.append(_p)

import numpy as np
import ml_dtypes

B, L, D, H = 2, 2048, 1024, 16
E = D // H          # 64 head dim
N_CORES = 8
HG = 4              # heads per core
CW = HG * E         # 256 cols per core
KT = D // 128       # 8 contraction k-tiles
NS = L // 128       # 16 key tiles
NCH = 4             # query chunks
CHW = L // NCH      # 512 queries per chunk
NQB = CHW // 128    # 4 q-blocks per chunk
SCALE = 1.0 / np.sqrt(E)

# Schraudolph exp on DVE for these key-tiles (rest on ACT); heads 0/1 are
# scored inside the tail window where DVE also runs norm/transpose-evac,
# so they get fewer DVE tiles.
DVE_KT_BY_H = {
    0: (1, 3, 5, 7, 9, 11, 13),
    1: (2, 4, 6, 8, 10, 12, 14),
    2: (0, 2, 4, 6, 8, 10, 12, 14, 15),
    3: (1, 3, 5, 7, 9, 11, 13, 15, 0),
}
SCH_A = 128.0 / np.log(2.0) * SCALE    # 23.083
WSCALE = 32.0   # qkv weights scaled up for fp8 hi-lo (denormal floor), undone at evac
SCH_B = 16250.5                        # 127*128 - 5.5 (centering) + 0.5 (trunc)

BF16 = ml_dtypes.bfloat16

_CACHED = {}
SPLIT_WAITS = True  # walrus HW build needs single-wait insts; CoreSim chokes on the split NoOps


def _patch_drain(tile_mod, mybir):
    """This walrus build only accepts one sync-wait on a Drain; spread the
    final Tile drain's waits over single-wait NOPs."""
    from concourse.vector_clock import ScopedClock

    def _dab(self, tick_clock, wait_clock):
        nc = self.nc
        drain_inst = nc.sync.drain()
        wait_clock.add_sem_waits(
            drain_inst.ins, ScopedClock({None: tick_clock.global_clock})
        )
        waits = list(drain_inst.ins.sync_info.on_wait)
        if len(waits) > 1:
            drain_inst.ins.sync_info.on_wait = waits[:1]
            for w in waits[1:]:
                nop = nc.sync.nop()
                if nop.ins.sync_info is None:
                    nop.ins.sync_info = mybir.SyncInfo(on_wait=[w], on_update=[])
                else:
                    nop.ins.sync_info.on_wait = [w]
        nc.all_engine_barrier()
        assert self.sems is not None
        popped = nc._tile_sem_poison_stack.pop()
        assert popped is self._sem_poison
        nc.clear_and_free_semaphores(list(self.sems.allocated().values()))
        nc.all_engine_barrier()

    tile_mod.TileContext._drain_and_barrier = _dab


def _split_multi_waits(nc, mybir):
    """One sync-wait per instruction; move extras onto same-engine NOPs."""
    ctr = 0
    for blk in nc.m.functions[0].blocks:
        insts = list(blk.instructions)
        out = []
        for inst in insts:
            si = getattr(inst, "sync_info", None)
            if si is not None and si.on_wait is not None and len(si.on_wait) > 1:
                waits = list(si.on_wait)
                for w in waits[:-1]:
                    nop = mybir.InstNoOp(
                        name=f"I-waitsplit-{ctr}",
                        engine=inst.engine,
                        sync_info=mybir.SyncInfo(on_wait=[w], on_update=[]),
                        bass_nofuse=True,
                    )
                    ctr += 1
                    out.append(nop)
                si.on_wait = waits[-1:]
            out.append(inst)
        if len(out) != len(insts):
            blk.instructions[:] = out


def _emit(nc, tile, mybir, ctx, tc, t):
    import concourse.bass as bass_mod

    f32 = mybir.dt.float32
    bf16 = mybir.dt.bfloat16
    i16 = mybir.dt.int16
    AF = mybir.ActivationFunctionType
    OP = mybir.AluOpType
    X = mybir.AxisListType.X

    consts = ctx.enter_context(tc.tile_pool(name="consts", bufs=1))
    dramp = ctx.enter_context(tc.tile_pool(name="dramp", bufs=2, space="DRAM"))

    # ---- SBUF constants / persistent tiles ----
    cb = consts.tile([128, 5 + CW], f32)
    nc.sync.dma_start(out=cb, in_=t["cb"])
    bq_t, bk_t = cb[:, 0:2], cb[:, 2:4]
    bf_c = cb[:, 4:5]
    bv_b = cb[:, 5:5 + CW]

    # all host-side tensors are packed [128, KT*cols] (k-tiles side by side);
    # x and qkv weights come as fp8 hi+lo pairs for DoubleRow hi-lo matmuls
    f8 = mybir.dt.float8e4
    xh_all = consts.tile([128, KT * L], f8, name="xh_all", tag="xh_all")
    xl_all = consts.tile([128, KT * L], f8, name="xl_all", tag="xl_all")
    xh = [xh_all[:, k * L:(k + 1) * L] for k in range(KT)]
    xl = [xl_all[:, k * L:(k + 1) * L] for k in range(KT)]

    def w_packed(name, cols, dt=bf16):
        all_t = consts.tile([128, KT * cols], dt, name=f"{name}_all",
                            tag=f"{name}_all")
        return all_t, [all_t[:, k * cols:(k + 1) * cols] for k in range(KT)]

    wqh_all, _ = w_packed("wqh", CW, f8)
    wql_all, _ = w_packed("wql", CW, f8)
    wkh_all, _ = w_packed("wkh", CW, f8)
    wkl_all, _ = w_packed("wkl", CW, f8)
    wvh_all, _ = w_packed("wvh", CW, f8)
    wvl_all, _ = w_packed("wvl", CW, f8)
    wg_all, wg = w_packed("wg", D)

    def dr3(big, off, mid, n):
        # [128, 2, n] DoubleRow AP into packed tile `big` at element offset
        return bass_mod.AP(tensor=big.tensor, offset=big.offset + off,
                           ap=[[big.ap[0][0], 128], [mid, 2], [1, n]])
    wfb = consts.tile([128, KT], bf16)
    ident = consts.tile([128, 128], bf16)
    bo4 = consts.tile([1, D], mybir.dt.float8e4)
    bgT = consts.tile([128, KT], f32)
    wo_all = consts.tile([128, 2 * D], bf16, name="wo_all", tag="wo_all")
    wo = [wo_all[:, k * D:(k + 1) * D] for k in range(2)]

    # few, large DMAs (HWDGE issue is a serialized ~630ns/DMA resource):
    # wv + wfb first, x hi/lo in 2-ktile pieces, then wq/wk, wo, wg.
    nc.sync.dma_start(out=wvh_all, in_=t["wvh"])
    nc.scalar.dma_start(out=wvl_all, in_=t["wvl"])
    nc.scalar.dma_start(out=wfb, in_=t["wfb"])
    for kp in range(KT // 2):
        nc.sync.dma_start(out=xh_all[:, 2 * kp * L:(2 * kp + 2) * L],
                          in_=t["xh"][:, 2 * kp * L:(2 * kp + 2) * L])
        nc.scalar.dma_start(out=xl_all[:, 2 * kp * L:(2 * kp + 2) * L],
                            in_=t["xl"][:, 2 * kp * L:(2 * kp + 2) * L])
    nc.scalar.dma_start(out=wqh_all, in_=t["wqh"])
    nc.scalar.dma_start(out=wql_all, in_=t["wql"])
    nc.scalar.dma_start(out=wkh_all, in_=t["wkh"])
    nc.scalar.dma_start(out=wkl_all, in_=t["wkl"])
    nc.sync.dma_start(out=wo_all, in_=t["wo"])
    nc.scalar.dma_start(out=ident, in_=t["ident"])
    nc.scalar.dma_start(out=bo4, in_=t["bo4"])
    nc.scalar.dma_start(out=bgT, in_=t["bgT"])
    for half in range(2):
        nc.sync.dma_start(out=wg_all[:, half * 4 * D:(half + 1) * 4 * D],
                          in_=t["wg"][:, half * 4 * D:(half + 1) * 4 * D])

    qt = [consts.tile([128, L], bf16, name=f"qt{i}", tag=f"qt{i}") for i in range(2)]
    kt_ = [consts.tile([128, L], bf16, name=f"kt{i}", tag=f"kt{i}") for i in range(2)]
    v_aug = [consts.tile([128, CW], bf16, name=f"va{i}", tag=f"va{i}")
             for i in range(NS)]
    ones_c = consts.tile([128, 1], bf16)
    vt = [consts.tile([128, L], bf16, name=f"vt{i}", tag=f"vt{i}") for i in range(2)]
    gate_t = consts.tile([128, NS], f32)
    gate_b = consts.tile([128, NS], mybir.dt.float8e4)
    omg_b = consts.tile([128, NS], mybir.dt.float8e4)
    gateomg = consts.tile([1, 2 * L], mybir.dt.float8e4)
    bovg = consts.tile([1, 2 * D], mybir.dt.float8e4)
    xsum = consts.tile([128, KT], f32)
    xsum_b = consts.tile([128, KT], bf16)
    vg4T = consts.tile([128, KT], mybir.dt.float8e4)

    nc.vector.memset(ones_c, 1.0)

    # ---- phase A: projections, gate, global context ----
    with tc.tile_pool(name="pav", bufs=2, space="PSUM") as pav, \
         tc.tile_pool(name="paq", bufs=2, space="PSUM") as paq, \
         tc.tile_pool(name="pag", bufs=1, space="PSUM") as pag:
        # V projection + per-tile evac with bias
        NKP = KT // 2
        DR = mybir.MatmulPerfMode.DoubleRow
        for s in range(NS):
            ps = pav.tile([128, CW], f32, name="vps", tag="vps")
            terms = ((xh_all, wvh_all), (xh_all, wvl_all), (xl_all, wvh_all))
            for ti, (xa, wa) in enumerate(terms):
                for kp in range(NKP):
                    nc.tensor.matmul(
                        out=ps,
                        lhsT=dr3(xa, 2 * kp * L + s * 128, L, 128),
                        rhs=dr3(wa, 2 * kp * CW, CW, CW),
                        start=(ti == 0 and kp == 0),
                        stop=(ti == 2 and kp == NKP - 1), perf_mode=DR)
            nc.vector.scalar_tensor_tensor(out=v_aug[s], in0=ps,
                                           scalar=1.0 / WSCALE, op0=OP.mult,
                                           in1=bv_b, op1=OP.add)
        # gate pre-activations via N=1 matmuls, one sigmoid
        gpre = pag.tile([128, NS], f32, name="gpre", tag="gpre")
        for s in range(NS):
            for xi, xa in enumerate((xh_all, xl_all)):
                for k in range(KT):
                    nc.tensor.matmul(
                        out=gpre[:, s:s + 1],
                        lhsT=bass_mod.AP(tensor=xa.tensor,
                                         offset=xa.offset + k * L + s * 128,
                                         ap=[[xa.ap[0][0], 128], [1, 128]]),
                        rhs=wfb[:, k:k + 1],
                        start=(xi == 0 and k == 0),
                        stop=(xi == 1 and k == KT - 1))
        nc.scalar.activation(gate_t, gpre, AF.Sigmoid, bias=bf_c)
        nc.vector.tensor_copy(gate_b, gate_t)
        nc.vector.tensor_scalar(out=omg_b, in0=gate_t, scalar1=-1.0, scalar2=1.0,
                                op0=OP.mult, op1=OP.add)
        gd = dramp.tile([1, L], mybir.dt.float8e4, name="gd", tag="gd")
        od = dramp.tile([1, L], mybir.dt.float8e4, name="od", tag="od")
        nc.sync.dma_start(out=gd, in_=gate_b)
        nc.sync.dma_start(out=od, in_=omg_b)
        nc.sync.dma_start(out=gateomg[:, 0:L], in_=bass_mod.AP(
            tensor=gd.tensor, offset=gd.offset, ap=[[0, 1], [1, NS], [NS, 128]]))
        nc.sync.dma_start(out=gateomg[:, L:2 * L], in_=bass_mod.AP(
            tensor=od.tensor, offset=od.offset, ap=[[0, 1], [1, NS], [NS, 128]]))
        nc.sync.dma_start(out=bovg[:, 0:D], in_=bo4)

        # Q/K projections -> qt/kt (bias evac split ACT/DVE)
        for c in range(2):
            for dst, wh, wl, bias in ((qt, wqh_all, wql_all, bq_t),
                                      (kt_, wkh_all, wkl_all, bk_t)):
                for lo in range(0, L, 512):
                    ps = paq.tile([128, 512], f32, name="qkps", tag="qkps")
                    terms = ((wh, xh_all), (wl, xh_all), (wh, xl_all))
                    for ti, (wa, xa) in enumerate(terms):
                        for kp in range(NKP):
                            nc.tensor.matmul(
                                out=ps,
                                lhsT=dr3(wa, 2 * kp * CW + c * 128, CW, 128),
                                rhs=dr3(xa, 2 * kp * L + lo, L, 512),
                                start=(ti == 0 and kp == 0),
                                stop=(ti == 2 and kp == NKP - 1), perf_mode=DR)
                    if lo % 1024 == 0:
                        nc.scalar.activation(dst[c][:, lo:lo + 512], ps,
                                             AF.Identity, bias=bias[:, c:c + 1],
                                             scale=1.0 / WSCALE)
                    else:
                        nc.vector.tensor_scalar(
                            out=dst[c][:, lo:lo + 512], in0=ps,
                            scalar1=1.0 / WSCALE, scalar2=bias[:, c:c + 1],
                            op0=OP.mult, op1=OP.add)

        # (global-context xsum/VG emitted later, inside chunk 0: emit_vg)

    # ---- phase B/C: attention + output projection, chunk-major ----
    ex_sb = ctx.enter_context(tc.tile_pool(name="ex_sb", bufs=2))
    avn_sb = ctx.enter_context(tc.tile_pool(name="avn_sb", bufs=3))
    rc_sb = ctx.enter_context(tc.tile_pool(name="rc_sb", bufs=2))
    y_sb = ctx.enter_context(tc.tile_pool(name="y_sb", bufs=3))
    # single PSUM pool, per-tag bufs: st x2 + avs x4 + tr x1 + yps x1 = 8 banks
    psb = ctx.enter_context(tc.tile_pool(name="psb", bufs=1, space="PSUM"))
    stp = avp = trp = yp = psb

    ex = {}     # (h) -> list of 16 ex tiles for current chunk
    avs = {}    # qb -> psum tile [128, HG*(E+1)]
    yd = t["y"].rearrange("(t p) d -> t p d", p=128)

    def emit_vg():
        xsum_scratch = consts.tile([128, L], bf16, name="xsum_scratch")
        xsuml = consts.tile([128, KT], f32, name="xsuml")
        for k in range(KT):
            nc.scalar.activation(xsum_scratch, xh[k], AF.Copy,
                                 accum_out=xsum[:, k:k + 1])
            nc.vector.reduce_sum(out=xsuml[:, k:k + 1], in_=xl[k], axis=X)
        nc.vector.tensor_add(xsum, xsum, xsuml)
        nc.vector.tensor_copy(xsum_b, xsum)
        vgT = yp.tile([128, KT], f32, name="vgT", tag="tail", bufs=2)
        for ch in range(KT):
            for k in range(KT):
                nc.tensor.matmul(out=vgT[:, ch:ch + 1],
                                 lhsT=wg[k][:, ch * 128:(ch + 1) * 128],
                                 rhs=xsum_b[:, k:k + 1],
                                 start=(k == 0), stop=(k == KT - 1))
        nc.vector.scalar_tensor_tensor(out=vg4T, in0=vgT, scalar=0.25 / L,
                                       op0=OP.mult, in1=bgT, op1=OP.add)
        vgd = dramp.tile([1, D], mybir.dt.float8e4, name="vgd", tag="vgd")
        nc.sync.dma_start(out=vgd, in_=vg4T)
        nc.sync.dma_start(out=bovg[:, D:2 * D], in_=bass_mod.AP(
            tensor=vgd.tensor, offset=vgd.offset, ap=[[0, 1], [1, KT], [KT, 128]]))

    def emit_scores(c, h):
        exl = []
        for s in range(NS):
            st = stp.tile([128, CHW], f32, name="st", tag="st", bufs=3)
            nc.tensor.matmul(
                out=st,
                lhsT=kt_[h // 2][64 * (h % 2):64 * (h % 2) + 64, s * 128:(s + 1) * 128],
                rhs=qt[h // 2][64 * (h % 2):64 * (h % 2) + 64, c * CHW:(c + 1) * CHW],
                start=True, stop=True)
            ext = ex_sb.tile([128, CHW], bf16, name="ex", tag=f"ex{s}", bufs=3)
            if s in DVE_KT_BY_H[h]:
                nc.vector.tensor_scalar(out=ext.bitcast(i16), in0=st,
                                        scalar1=SCH_A, scalar2=SCH_B,
                                        op0=OP.mult, op1=OP.add)
            else:
                nc.scalar.activation(ext, st, AF.Exp, scale=SCALE)
            exl.append(ext)
        ex[h] = exl

    def emit_av(c, h):
        if h == 0:
            avs[0] = avp.tile([128, 2 * CW], f32, name="avsA", tag="avsA")
            avs[1] = avp.tile([128, 2 * CW], f32, name="avsB", tag="avsB")
            avs["d"] = avp.tile([128, HG * NQB], f32, name="avsd", tag="avsd")
        for qb in range(NQB):
            lo = (qb % 2) * CW + h * E
            dcol = h * NQB + qb
            for s in range(NS):
                exsl = ex[h][s][:, qb * 128:(qb + 1) * 128]
                nc.tensor.matmul(
                    out=avs[qb // 2][:, lo:lo + E],
                    lhsT=exsl, rhs=v_aug[s][:, h * E:(h + 1) * E],
                    start=(s == 0), stop=(s == NS - 1))
                nc.tensor.matmul(
                    out=avs["d"][:, dcol:dcol + 1],
                    lhsT=exsl, rhs=ones_c,
                    start=(s == 0), stop=(s == NS - 1))

    def emit_norm(c):
        # normalize + transpose into vt for chunk c
        for qb in range(NQB):
            lt = c * NQB + qb
            d4 = avs["d"].rearrange("p (h q) -> p h q", q=NQB)[:, :, qb]
            rc4 = rc_sb.tile([128, HG], f32, name="rc4", tag="rc4")
            nc.vector.reciprocal(rc4, d4)
            nc.vector.tensor_scalar(out=rc4, in0=rc4,
                                    scalar1=gate_t[:, lt:lt + 1], scalar2=None,
                                    op0=OP.mult)
            avn = avn_sb.tile([128, HG * E], bf16, name="avn", tag="avn")
            rc_b = bass_mod.AP(tensor=rc4.tensor, offset=rc4.offset,
                               ap=[[rc4.ap[0][0], 128], [1, HG], [0, E]])
            src_ap = avs[qb // 2][:, (qb % 2) * CW:(qb % 2) * CW + CW]
            nc.vector.tensor_tensor(
                out=avn.rearrange("p (h c) -> p h c", c=E),
                in0=src_ap.rearrange("p (h c) -> p h c", c=E),
                in1=rc_b, op=OP.mult)
            for kc in range(2):
                tr = trp.tile([128, 128], bf16, name="tr", tag="tail", bufs=2)
                nc.tensor.transpose(tr, avn[:, kc * 128:(kc + 1) * 128], ident)
                nc.scalar.activation(vt[kc][:, lt * 128:(lt + 1) * 128], tr,
                                     AF.Copy)

    def emit_wo(c):
        # output projection + bias fusion + evac + DMA for chunk c
        for qb in range(NQB):
            lt = c * NQB + qb
            lsl = slice(lt * 128, (lt + 1) * 128)
            for do in range(0, D, 512):
                ps = yp.tile([128, 512], f32, name="yps", tag="tail", bufs=2)
                nc.tensor.matmul(out=ps, lhsT=vt[0][:, lsl],
                                 rhs=wo[0][:, do:do + 512], start=True, stop=False)
                nc.tensor.matmul(out=ps, lhsT=vt[1][:, lsl],
                                 rhs=wo[1][:, do:do + 512], start=False, stop=False)
                gdr = bass_mod.AP(tensor=gateomg.tensor, offset=gateomg.offset
                                  + lt * 128, ap=[[gateomg.ap[0][0], 1], [L, 2],
                                                  [1, 128]])
                bdr = bass_mod.AP(tensor=bovg.tensor, offset=bovg.offset + do,
                                  ap=[[bovg.ap[0][0], 1], [D, 2], [1, 512]])
                nc.tensor.matmul(out=ps, lhsT=gdr, rhs=bdr, start=False,
                                 stop=True, perf_mode=mybir.MatmulPerfMode.DoubleRow)
                ot = y_sb.tile([128, 512], bf16, name="ot", tag="ot")
                if do == 0:
                    nc.scalar.activation(ot, ps, AF.Copy)
                else:
                    nc.vector.tensor_copy(ot, ps)
                nc.sync.dma_start(out=yd[lt][:, do:do + 512], in_=ot)

    # interleaved emission: scores run ahead of AV; each chunk's norm and
    # output projection slot between the next chunk's first score blocks so
    # no engine head-of-line blocks.
    for c in range(NCH):
        if c == 0:
            emit_scores(0, 0)
            emit_scores(0, 1)
        emit_av(c, 0)
        if c == 0:
            emit_vg()
        emit_scores(c, 2)
        emit_av(c, 1)
        emit_scores(c, 3)
        emit_av(c, 2)
        if c + 1 < NCH:
            emit_scores(c + 1, 0)
        emit_av(c, 3)
        if c + 1 < NCH:
            emit_scores(c + 1, 1)
        emit_norm(c)
        emit_wo(c)


def _emit_main(nc, tile, mybir, ctx, tc, t):
    _emit(nc, tile, mybir, ctx, tc, t)


def _build():
    if "nc" in _CACHED:
        return _CACHED["nc"]
    import concourse.bass as bass
    import concourse.tile as tile
    from concourse import mybir
    from contextlib import ExitStack

    _patch_drain(tile, mybir)
    nc = bass.Bass("TRN2", target_bir_lowering=False, debug=False)
    f32, bf16 = mybir.dt.float32, mybir.dt.bfloat16
    t = {
        "xh": nc.dram_tensor("xh", [128, KT * L], mybir.dt.float8e4, kind="ExternalInput").ap(),
        "xl": nc.dram_tensor("xl", [128, KT * L], mybir.dt.float8e4, kind="ExternalInput").ap(),
        "cb": nc.dram_tensor("cb", [128, 5 + CW], f32, kind="ExternalInput").ap(),
        "wqh": nc.dram_tensor("wqh", [128, KT * CW], mybir.dt.float8e4, kind="ExternalInput").ap(),
        "wql": nc.dram_tensor("wql", [128, KT * CW], mybir.dt.float8e4, kind="ExternalInput").ap(),
        "wkh": nc.dram_tensor("wkh", [128, KT * CW], mybir.dt.float8e4, kind="ExternalInput").ap(),
        "wkl": nc.dram_tensor("wkl", [128, KT * CW], mybir.dt.float8e4, kind="ExternalInput").ap(),
        "wvh": nc.dram_tensor("wvh", [128, KT * CW], mybir.dt.float8e4, kind="ExternalInput").ap(),
        "wvl": nc.dram_tensor("wvl", [128, KT * CW], mybir.dt.float8e4, kind="ExternalInput").ap(),
        "wfb": nc.dram_tensor("wfb", [128, KT], bf16, kind="ExternalInput").ap(),
        "wo": nc.dram_tensor("wo", [128, 2 * D], bf16, kind="ExternalInput").ap(),
        "wg": nc.dram_tensor("wg", [128, KT * D], bf16, kind="ExternalInput").ap(),
        "ident": nc.dram_tensor("ident", [128, 128], bf16, kind="ExternalInput").ap(),
        "bo4": nc.dram_tensor("bo4", [1, D], mybir.dt.float8e4, kind="ExternalInput").ap(),
        "bgT": nc.dram_tensor("bgT", [128, KT], f32, kind="ExternalInput").ap(),
        "y": nc.dram_tensor("y", [L, D], bf16, kind="ExternalOutput").ap(),
    }
    with tile.TileContext(nc) as tc:
        with ExitStack() as ctx:
            _emit_main(nc, tile, mybir, ctx, tc, t)
    if SPLIT_WAITS:
        _split_multi_waits(nc, mybir)
    _CACHED["nc"] = nc
    return nc


def _pack(w):
    """[KT*128, C] -> [128, KT*C] with k-tiles side by side."""
    kt = w.shape[0] // 128
    return np.ascontiguousarray(
        w.reshape(kt, 128, w.shape[1]).transpose(1, 0, 2).reshape(128, -1))


F8 = ml_dtypes.float8_e4m3fn


def _hilo(a):
    hi = a.astype(F8)
    lo = (a - hi.astype(np.float32)).astype(F8)
    return hi, lo


def _prep_core_inputs(c, inputs, shared):
    b, g = c // 4, c % 4
    cols = slice(g * CW, (g + 1) * CW)
    bf_val = float(np.asarray(inputs["bf"]).reshape(-1)[0])
    wqh, wql = _hilo(_pack(inputs["Wq"][:, cols]) * WSCALE)
    wkh, wkl = _hilo(_pack(inputs["Wk"][:, cols]) * WSCALE)
    wvh, wvl = _hilo(_pack(inputs["Wv"][:, cols]) * WSCALE)
    m = {
        "xh": shared["xh"][b], "xl": shared["xl"][b],
        "wqh": wqh, "wql": wql, "wkh": wkh, "wkl": wkl,
        "wvh": wvh, "wvl": wvl,
        "wfb": shared["wfb"],
        "wo": _pack(inputs["Wo"][cols, :]).astype(BF16),
        "wg": shared["wg"],
        "ident": shared["ident"],
        "cb": np.concatenate([
            inputs["bq"][cols].reshape(2, 128).T,
            inputs["bk"][cols].reshape(2, 128).T,
            np.full((128, 1), bf_val, np.float32),
            np.broadcast_to(inputs["bv"][cols][None, :], (128, CW)),
        ], axis=1).astype(np.float32),
        "bo4": (inputs["bo"][None, :] * 0.25).astype(ml_dtypes.float8_e4m3fn),
        "bgT": (inputs["bg"].reshape(KT, 128).T * 0.25).astype(np.float32),
    }
    return m


def kernel(**inputs):
    from concourse import bass_utils

    nc = _build()
    xhl = [_hilo(_pack(inputs["x"][b].T)) for b in range(B)]
    shared = {
        "xh": [xhl[b][0] for b in range(B)],
        "xl": [xhl[b][1] for b in range(B)],
        "wg": _pack(inputs["Wg"]).astype(BF16),
        "wfb": np.ascontiguousarray(inputs["Wf"].reshape(KT, 128).T).astype(BF16),
        "ident": np.eye(128, dtype=np.float32).astype(BF16),
    }
    in_maps = [_prep_core_inputs(c, inputs, shared) for c in range(N_CORES)]
    res = bass_utils.run_bass_kernel_spmd(nc, in_maps, core_ids=list(range(N_CORES)))
    out = np.zeros((B, L, D), np.float32)
    for c in range(N_CORES):
        out[c // 4] += res.results[c]["y"]
    return out


# revision 6
# speedup vs baseline: 1.1144x; 1.0115x over previous
"""Trainium2 Bass kernel v2 for the gated-attention layer.

Sharding: 8 cores = (2 batches) x (4 head-groups of 4 heads each), as v1.

Key structure changes vs v1:
- Flipped AV matmul: out[q, head_dim] = ex_tile.T @ v_slice with N=65 per
  matmul (cost-model charges by output free size) -> AV cost halves, and
  denominators land per-partition (no DMA-bounce transposes).
- VG and gate pre-activations via N=1 matmuls (out [128,1] per k-tile).
- exp split between ScalarE (exact) and DVE (Schraudolph int16->bf16
  bitcast) so ACT is not the attention bottleneck.
- chunk-major loop (512 queries) with interleaved emission so PE never
  head-of-line blocks on exp.
- Output projection per chunk; y evacuated on ACT, DMA'd per 512-col tile.
"""

import sys

for _p in ("/root/.axon_site/_ro/trn_rl_repo", "/opt/trn_rl_repo"):
    if _p not in sys.path:
        sys.path.append(_p)

import numpy as np
import ml_dtypes

B, L, D, H = 2, 2048, 1024, 16
E = D // H          # 64 head dim
N_CORES = 8
HG = 4              # heads per core
CW = HG * E         # 256 cols per core
KT = D // 128       # 8 contraction k-tiles
NS = L // 128       # 16 key tiles
NCH = 4             # query chunks
CHW = L // NCH      # 512 queries per chunk
NQB = CHW // 128    # 4 q-blocks per chunk
SCALE = 1.0 / np.sqrt(E)

# Schraudolph exp on DVE for these key-tiles (rest on ACT); heads 0/1 are
# scored inside the tail window where DVE also runs norm/transpose-evac,
# so they get fewer DVE tiles.
DVE_KT_BY_H = {
    0: (1, 3, 5, 7, 9, 11, 13),
    1: (2, 4, 6, 8, 10, 12, 14),
    2: (0, 2, 4, 6, 8, 10, 12, 14, 15),
    3: (1, 3, 5, 7, 9, 11, 13, 15, 0),
}
SCH_A = 128.0 / np.log(2.0) * SCALE    # 23.083
WSCALE = 32.0   # qkv weights scaled up for fp8 hi-lo (denormal floor), undone at evac
SCH_B = 16250.5                        # 127*128 - 5.5 (centering) + 0.5 (trunc)

BF16 = ml_dtypes.bfloat16

_CACHED = {}
SPLIT_WAITS = True  # walrus HW build needs single-wait insts; CoreSim chokes on the split NoOps


def _patch_drain(tile_mod, mybir):
    """This walrus build only accepts one sync-wait on a Drain; spread the
    final Tile drain's waits over single-wait NOPs."""
    from concourse.vector_clock import ScopedClock

    def _dab(self, tick_clock, wait_clock):
        nc = self.nc
        drain_inst = nc.sync.drain()
        wait_clock.add_sem_waits(
            drain_inst.ins, ScopedClock({None: tick_clock.global_clock})
        )
        waits = list(drain_inst.ins.sync_info.on_wait)
        if len(waits) > 1:
            drain_inst.ins.sync_info.on_wait = waits[:1]
            for w in waits[1:]:
                nop = nc.sync.nop()
                if nop.ins.sync_info is None:
                    nop.ins.sync_info = mybir.SyncInfo(on_wait=[w], on_update=[])
                else:
                    nop.ins.sync_info.on_wait = [w]
        nc.all_engine_barrier()
        assert self.sems is not None
        popped = nc._tile_sem_poison_stack.pop()
        assert popped is self._sem_poison
        nc.clear_and_free_semaphores(list(self.sems.allocated().values()))
        nc.all_engine_barrier()

    tile_mod.TileContext._drain_and_barrier = _dab


def _split_multi_waits(nc, mybir):
    """One sync-wait per instruction; move extras onto same-engine NOPs."""
    ctr = 0
    for blk in nc.m.functions[0].blocks:
        insts = list(blk.instructions)
        out = []
        for inst in insts:
            si = getattr(inst, "sync_info", None)
            if si is not None and si.on_wait is not None and len(si.on_wait) > 1:
                waits = list(si.on_wait)
                for w in waits[:-1]:
                    nop = mybir.InstNoOp(
                        name=f"I-waitsplit-{ctr}",
                        engine=inst.engine,
                        sync_info=mybir.SyncInfo(on_wait=[w], on_update=[]),
                        bass_nofuse=True,
                    )
                    ctr += 1
                    out.append(nop)
                si.on_wait = waits[-1:]
            out.append(inst)
        if len(out) != len(insts):
            blk.instructions[:] = out


def _emit(nc, tile, mybir, ctx, tc, t):
    import concourse.bass as bass_mod

    f32 = mybir.dt.float32
    bf16 = mybir.dt.bfloat16
    i16 = mybir.dt.int16
    AF = mybir.ActivationFunctionType
    OP = mybir.AluOpType
    X = mybir.AxisListType.X

    consts = ctx.enter_context(tc.tile_pool(name="consts", bufs=1))
    dramp = ctx.enter_context(tc.tile_pool(name="dramp", bufs=2, space="DRAM"))

    # ---- SBUF constants / persistent tiles ----
    cb = consts.tile([128, 5 + CW], f32)
    nc.sync.dma_start(out=cb, in_=t["cb"])
    bq_t, bk_t = cb[:, 0:2], cb[:, 2:4]
    bf_c = cb[:, 4:5]
    bv_b = cb[:, 5:5 + CW]

    # all host-side tensors are packed [128, KT*cols] (k-tiles side by side);
    # x and qkv weights come as fp8 hi+lo pairs for DoubleRow hi-lo matmuls
    f8 = mybir.dt.float8e4
    xh_all = consts.tile([128, KT * L], f8, name="xh_all", tag="xh_all")
    xl_all = consts.tile([128, KT * L], f8, name="xl_all", tag="xl_all")
    xh = [xh_all[:, k * L:(k + 1) * L] for k in range(KT)]
    xl = [xl_all[:, k * L:(k + 1) * L] for k in range(KT)]

    def w_packed(name, cols, dt=bf16):
        all_t = consts.tile([128, KT * cols], dt, name=f"{name}_all",
                            tag=f"{name}_all")
        return all_t, [all_t[:, k * cols:(k + 1) * cols] for k in range(KT)]

    wqh_all, _ = w_packed("wqh", CW, f8)
    wql_all, _ = w_packed("wql", CW, f8)
    wkh_all, _ = w_packed("wkh", CW, f8)
    wkl_all, _ = w_packed("wkl", CW, f8)
    wvh_all, _ = w_packed("wvh", CW, f8)
    wvl_all, _ = w_packed("wvl", CW, f8)
    wg_all, wg = w_packed("wg", D)

    def dr3(big, off, mid, n):
        # [128, 2, n] DoubleRow AP into packed tile `big` at element offset
        return bass_mod.AP(tensor=big.tensor, offset=big.offset + off,
                           ap=[[big.ap[0][0], 128], [mid, 2], [1, n]])
    wfb = consts.tile([128, KT], bf16)
    ident = consts.tile([128, 128], bf16)
    bo4 = consts.tile([1, D], mybir.dt.float8e4)
    bgT = consts.tile([128, KT], f32)
    wo_all = consts.tile([128, 2 * D], bf16, name="wo_all", tag="wo_all")
    wo = [wo_all[:, k * D:(k + 1) * D] for k in range(2)]

    # few, large DMAs (HWDGE issue is a serialized ~630ns/DMA resource):
    # wv + wfb first, x hi/lo in 2-ktile pieces, then wq/wk, wo, wg.
    nc.sync.dma_start(out=wvh_all, in_=t["wvh"])
    nc.scalar.dma_start(out=wvl_all, in_=t["wvl"])
    nc.scalar.dma_start(out=wfb, in_=t["wfb"])

    def x_piece(sb_tile, dram_ap, lc):
        # all KT k-tiles, L-columns [lc*512, (lc+1)*512): lets the s-loop
        # start after the first piece instead of after the whole tensor
        mk = lambda ap, off: bass_mod.AP(
            tensor=ap.tensor, offset=ap.offset + off,
            ap=[[ap.ap[0][0], 128], [L, KT], [1, 512]])
        return mk(sb_tile, lc * 512), mk(dram_ap, lc * 512)

    for lc in range(4):
        o, i = x_piece(xh_all, t["xh"], lc)
        nc.sync.dma_start(out=o, in_=i)
        o, i = x_piece(xl_all, t["xl"], lc)
        nc.scalar.dma_start(out=o, in_=i)
    nc.scalar.dma_start(out=wqh_all, in_=t["wqh"])
    nc.scalar.dma_start(out=wql_all, in_=t["wql"])
    nc.scalar.dma_start(out=wkh_all, in_=t["wkh"])
    nc.scalar.dma_start(out=wkl_all, in_=t["wkl"])
    nc.sync.dma_start(out=wo_all, in_=t["wo"])
    nc.scalar.dma_start(out=ident, in_=t["ident"])
    nc.scalar.dma_start(out=bo4, in_=t["bo4"])
    nc.scalar.dma_start(out=bgT, in_=t["bgT"])
    for half in range(2):
        nc.sync.dma_start(out=wg_all[:, half * 4 * D:(half + 1) * 4 * D],
                          in_=t["wg"][:, half * 4 * D:(half + 1) * 4 * D])

    qt = [consts.tile([128, L], bf16, name=f"qt{i}", tag=f"qt{i}") for i in range(2)]
    kt_ = [consts.tile([128, L], bf16, name=f"kt{i}", tag=f"kt{i}") for i in range(2)]
    v_aug = [consts.tile([128, CW], bf16, name=f"va{i}", tag=f"va{i}")
             for i in range(NS)]
    ones_c = consts.tile([128, 1], bf16)
    vt = [consts.tile([128, L], bf16, name=f"vt{i}", tag=f"vt{i}") for i in range(2)]
    gate_t = consts.tile([128, NS], f32)
    gate_b = consts.tile([128, NS], mybir.dt.float8e4)
    omg_b = consts.tile([128, NS], mybir.dt.float8e4)
    gateomg = consts.tile([1, 2 * L], mybir.dt.float8e4)
    bovg = consts.tile([1, 2 * D], mybir.dt.float8e4)
    xsum = consts.tile([128, KT], f32)
    xsum_b = consts.tile([128, KT], bf16)
    vg4T = consts.tile([128, KT], mybir.dt.float8e4)

    nc.vector.memset(ones_c, 1.0)

    # ---- phase A: projections, gate, global context ----
    with tc.tile_pool(name="pav", bufs=2, space="PSUM") as pav, \
         tc.tile_pool(name="paq", bufs=2, space="PSUM") as paq, \
         tc.tile_pool(name="pag", bufs=1, space="PSUM") as pag:
        # V projection + per-tile evac with bias
        NKP = KT // 2
        DR = mybir.MatmulPerfMode.DoubleRow
        for s in range(NS):
            ps = pav.tile([128, CW], f32, name="vps", tag="vps")
            terms = ((xh_all, wvh_all), (xh_all, wvl_all), (xl_all, wvh_all))
            for ti, (xa, wa) in enumerate(terms):
                for kp in range(NKP):
                    nc.tensor.matmul(
                        out=ps,
                        lhsT=dr3(xa, 2 * kp * L + s * 128, L, 128),
                        rhs=dr3(wa, 2 * kp * CW, CW, CW),
                        start=(ti == 0 and kp == 0),
                        stop=(ti == 2 and kp == NKP - 1), perf_mode=DR)
            nc.vector.scalar_tensor_tensor(out=v_aug[s], in0=ps,
                                           scalar=1.0 / WSCALE, op0=OP.mult,
                                           in1=bv_b, op1=OP.add)
        # gate pre-activations via N=1 matmuls, one sigmoid
        gpre = pag.tile([128, NS], f32, name="gpre", tag="gpre")
        for s in range(NS):
            for xi, xa in enumerate((xh_all, xl_all)):
                for k in range(KT):
                    nc.tensor.matmul(
                        out=gpre[:, s:s + 1],
                        lhsT=bass_mod.AP(tensor=xa.tensor,
                                         offset=xa.offset + k * L + s * 128,
                                         ap=[[xa.ap[0][0], 128], [1, 128]]),
                        rhs=wfb[:, k:k + 1],
                        start=(xi == 0 and k == 0),
                        stop=(xi == 1 and k == KT - 1))
        nc.scalar.activation(gate_t, gpre, AF.Sigmoid, bias=bf_c)
        nc.vector.tensor_copy(gate_b, gate_t)
        nc.vector.tensor_scalar(out=omg_b, in0=gate_t, scalar1=-1.0, scalar2=1.0,
                                op0=OP.mult, op1=OP.add)
        gd = dramp.tile([1, L], mybir.dt.float8e4, name="gd", tag="gd")
        od = dramp.tile([1, L], mybir.dt.float8e4, name="od", tag="od")
        nc.sync.dma_start(out=gd, in_=gate_b)
        nc.sync.dma_start(out=od, in_=omg_b)
        nc.sync.dma_start(out=gateomg[:, 0:L], in_=bass_mod.AP(
            tensor=gd.tensor, offset=gd.offset, ap=[[0, 1], [1, NS], [NS, 128]]))
        nc.sync.dma_start(out=gateomg[:, L:2 * L], in_=bass_mod.AP(
            tensor=od.tensor, offset=od.offset, ap=[[0, 1], [1, NS], [NS, 128]]))
        nc.sync.dma_start(out=bovg[:, 0:D], in_=bo4)

        # Q/K projections -> qt/kt (bias evac split ACT/DVE)
        for c in range(2):
            for dst, wh, wl, bias in ((qt, wqh_all, wql_all, bq_t),
                                      (kt_, wkh_all, wkl_all, bk_t)):
                for lo in range(0, L, 512):
                    ps = paq.tile([128, 512], f32, name="qkps", tag="qkps")
                    terms = ((wh, xh_all), (wl, xh_all), (wh, xl_all))
                    for ti, (wa, xa) in enumerate(terms):
                        for kp in range(NKP):
                            nc.tensor.matmul(
                                out=ps,
                                lhsT=dr3(wa, 2 * kp * CW + c * 128, CW, 128),
                                rhs=dr3(xa, 2 * kp * L + lo, L, 512),
                                start=(ti == 0 and kp == 0),
                                stop=(ti == 2 and kp == NKP - 1), perf_mode=DR)
                    if lo % 1024 == 0:
                        nc.scalar.activation(dst[c][:, lo:lo + 512], ps,
                                             AF.Identity, bias=bias[:, c:c + 1],
                                             scale=1.0 / WSCALE)
                    else:
                        nc.vector.tensor_scalar(
                            out=dst[c][:, lo:lo + 512], in0=ps,
                            scalar1=1.0 / WSCALE, scalar2=bias[:, c:c + 1],
                            op0=OP.mult, op1=OP.add)

        # (global-context xsum/VG emitted later, inside chunk 0: emit_vg)

    # ---- phase B/C: attention + output projection, chunk-major ----
    ex_sb = ctx.enter_context(tc.tile_pool(name="ex_sb", bufs=2))
    avn_sb = ctx.enter_context(tc.tile_pool(name="avn_sb", bufs=3))
    rc_sb = ctx.enter_context(tc.tile_pool(name="rc_sb", bufs=2))
    y_sb = ctx.enter_context(tc.tile_pool(name="y_sb", bufs=3))
    # single PSUM pool, per-tag bufs: st x2 + avs x4 + tr x1 + yps x1 = 8 banks
    psb = ctx.enter_context(tc.tile_pool(name="psb", bufs=1, space="PSUM"))
    stp = avp = trp = yp = psb

    ex = {}     # (h) -> list of 16 ex tiles for current chunk
    avs = {}    # qb -> psum tile [128, HG*(E+1)]
    yd = t["y"].rearrange("(t p) d -> t p d", p=128)

    def emit_vg():
        xsum_scratch = consts.tile([128, L], bf16, name="xsum_scratch")
        xsuml = consts.tile([128, KT], f32, name="xsuml")
        for k in range(KT):
            nc.scalar.activation(xsum_scratch, xh[k], AF.Copy,
                                 accum_out=xsum[:, k:k + 1])
            nc.vector.reduce_sum(out=xsuml[:, k:k + 1], in_=xl[k], axis=X)
        nc.vector.tensor_add(xsum, xsum, xsuml)
        nc.vector.tensor_copy(xsum_b, xsum)
        vgT = yp.tile([128, KT], f32, name="vgT", tag="tail", bufs=2)
        for ch in range(KT):
            for k in range(KT):
                nc.tensor.matmul(out=vgT[:, ch:ch + 1],
                                 lhsT=wg[k][:, ch * 128:(ch + 1) * 128],
                                 rhs=xsum_b[:, k:k + 1],
                                 start=(k == 0), stop=(k == KT - 1))
        nc.vector.scalar_tensor_tensor(out=vg4T, in0=vgT, scalar=0.25 / L,
                                       op0=OP.mult, in1=bgT, op1=OP.add)
        vgd = dramp.tile([1, D], mybir.dt.float8e4, name="vgd", tag="vgd")
        nc.sync.dma_start(out=vgd, in_=vg4T)
        nc.sync.dma_start(out=bovg[:, D:2 * D], in_=bass_mod.AP(
            tensor=vgd.tensor, offset=vgd.offset, ap=[[0, 1], [1, KT], [KT, 128]]))

    def emit_scores(c, h):
        dve_set = DVE_KT_BY_H[h]
        exl = []
        for s in range(NS):
            st = stp.tile([128, CHW], f32, name="st", tag="st", bufs=3)
            nc.tensor.matmul(
                out=st,
                lhsT=kt_[h // 2][64 * (h % 2):64 * (h % 2) + 64, s * 128:(s + 1) * 128],
                rhs=qt[h // 2][64 * (h % 2):64 * (h % 2) + 64, c * CHW:(c + 1) * CHW],
                start=True, stop=True)
            ext = ex_sb.tile([128, CHW], bf16, name="ex", tag=f"ex{s}", bufs=3)
            if s in dve_set:
                nc.vector.tensor_scalar(out=ext.bitcast(i16), in0=st,
                                        scalar1=SCH_A, scalar2=SCH_B,
                                        op0=OP.mult, op1=OP.add)
            else:
                nc.scalar.activation(ext, st, AF.Exp, scale=SCALE)
            exl.append(ext)
        ex[h] = exl

    def emit_av(c, h):
        if h == 0:
            avs[0] = avp.tile([128, 2 * CW], f32, name="avsA", tag="avsA")
            avs[1] = avp.tile([128, 2 * CW], f32, name="avsB", tag="avsB")
            avs["d"] = avp.tile([128, HG * NQB], f32, name="avsd", tag="avsd")
        for qb in range(NQB):
            lo = (qb % 2) * CW + h * E
            dcol = h * NQB + qb
            for s in range(NS):
                exsl = ex[h][s][:, qb * 128:(qb + 1) * 128]
                nc.tensor.matmul(
                    out=avs[qb // 2][:, lo:lo + E],
                    lhsT=exsl, rhs=v_aug[s][:, h * E:(h + 1) * E],
                    start=(s == 0), stop=(s == NS - 1))
                nc.tensor.matmul(
                    out=avs["d"][:, dcol:dcol + 1],
                    lhsT=exsl, rhs=ones_c,
                    start=(s == 0), stop=(s == NS - 1))

    def emit_norm(c):
        # normalize + transpose into vt for chunk c
        for qb in range(NQB):
            lt = c * NQB + qb
            d4 = avs["d"].rearrange("p (h q) -> p h q", q=NQB)[:, :, qb]
            rc4 = rc_sb.tile([128, HG], f32, name="rc4", tag="rc4")
            nc.vector.reciprocal(rc4, d4)
            nc.vector.tensor_scalar(out=rc4, in0=rc4,
                                    scalar1=gate_t[:, lt:lt + 1], scalar2=None,
                                    op0=OP.mult)
            avn = avn_sb.tile([128, HG * E], bf16, name="avn", tag="avn")
            rc_b = bass_mod.AP(tensor=rc4.tensor, offset=rc4.offset,
                               ap=[[rc4.ap[0][0], 128], [1, HG], [0, E]])
            src_ap = avs[qb // 2][:, (qb % 2) * CW:(qb % 2) * CW + CW]
            nc.vector.tensor_tensor(
                out=avn.rearrange("p (h c) -> p h c", c=E),
                in0=src_ap.rearrange("p (h c) -> p h c", c=E),
                in1=rc_b, op=OP.mult)
            for kc in range(2):
                tr = trp.tile([128, 128], bf16, name="tr", tag="tail", bufs=2)
                nc.tensor.transpose(tr, avn[:, kc * 128:(kc + 1) * 128], ident)
                nc.scalar.activation(vt[kc][:, lt * 128:(lt + 1) * 128], tr,
                                     AF.Copy)

    def emit_wo(c):
        # output projection + bias fusion + evac + DMA for chunk c
        for qb in range(NQB):
            lt = c * NQB + qb
            lsl = slice(lt * 128, (lt + 1) * 128)
            for do in range(0, D, 512):
                ps = yp.tile([128, 512], f32, name="yps", tag="tail", bufs=2)
                nc.tensor.matmul(out=ps, lhsT=vt[0][:, lsl],
                                 rhs=wo[0][:, do:do + 512], start=True, stop=False)
                nc.tensor.matmul(out=ps, lhsT=vt[1][:, lsl],
                                 rhs=wo[1][:, do:do + 512], start=False, stop=False)
                gdr = bass_mod.AP(tensor=gateomg.tensor, offset=gateomg.offset
                                  + lt * 128, ap=[[gateomg.ap[0][0], 1], [L, 2],
                                                  [1, 128]])
                bdr = bass_mod.AP(tensor=bovg.tensor, offset=bovg.offset + do,
                                  ap=[[bovg.ap[0][0], 1], [D, 2], [1, 512]])
                nc.tensor.matmul(out=ps, lhsT=gdr, rhs=bdr, start=False,
                                 stop=True, perf_mode=mybir.MatmulPerfMode.DoubleRow)
                ot = y_sb.tile([128, 512], bf16, name="ot", tag="ot")
                if do == 0:
                    nc.scalar.activation(ot, ps, AF.Copy)
                else:
                    nc.vector.tensor_copy(ot, ps)
                nc.sync.dma_start(out=yd[lt][:, do:do + 512], in_=ot)

    # interleaved emission: scores run ahead of AV; each chunk's norm and
    # output projection slot between the next chunk's first score blocks so
    # no engine head-of-line blocks.
    for c in range(NCH):
        if c == 0:
            emit_scores(0, 0)
            emit_scores(0, 1)
        emit_av(c, 0)
        if c == 0:
            emit_vg()
        emit_scores(c, 2)
        emit_av(c, 1)
        emit_scores(c, 3)
        emit_av(c, 2)
        if c + 1 < NCH:
            emit_scores(c + 1, 0)
        emit_av(c, 3)
        if c + 1 < NCH:
            emit_scores(c + 1, 1)
        emit_norm(c)
        emit_wo(c)


def _emit_main(nc, tile, mybir, ctx, tc, t):
    _emit(nc, tile, mybir, ctx, tc, t)


def _build():
    if "nc" in _CACHED:
        return _CACHED["nc"]
    import concourse.bass as bass
    import concourse.tile as tile
    from concourse import mybir
    from contextlib import ExitStack

    _patch_drain(tile, mybir)
    nc = bass.Bass("TRN2", target_bir_lowering=False, debug=False)
    f32, bf16 = mybir.dt.float32, mybir.dt.bfloat16
    t = {
        "xh": nc.dram_tensor("xh", [128, KT * L], mybir.dt.float8e4, kind="ExternalInput").ap(),
        "xl": nc.dram_tensor("xl", [128, KT * L], mybir.dt.float8e4, kind="ExternalInput").ap(),
        "cb": nc.dram_tensor("cb", [128, 5 + CW], f32, kind="ExternalInput").ap(),
        "wqh": nc.dram_tensor("wqh", [128, KT * CW], mybir.dt.float8e4, kind="ExternalInput").ap(),
        "wql": nc.dram_tensor("wql", [128, KT * CW], mybir.dt.float8e4, kind="ExternalInput").ap(),
        "wkh": nc.dram_tensor("wkh", [128, KT * CW], mybir.dt.float8e4, kind="ExternalInput").ap(),
        "wkl": nc.dram_tensor("wkl", [128, KT * CW], mybir.dt.float8e4, kind="ExternalInput").ap(),
        "wvh": nc.dram_tensor("wvh", [128, KT * CW], mybir.dt.float8e4, kind="ExternalInput").ap(),
        "wvl": nc.dram_tensor("wvl", [128, KT * CW], mybir.dt.float8e4, kind="ExternalInput").ap(),
        "wfb": nc.dram_tensor("wfb", [128, KT], bf16, kind="ExternalInput").ap(),
        "wo": nc.dram_tensor("wo", [128, 2 * D], bf16, kind="ExternalInput").ap(),
        "wg": nc.dram_tensor("wg", [128, KT * D], bf16, kind="ExternalInput").ap(),
        "ident": nc.dram_tensor("ident", [128, 128], bf16, kind="ExternalInput").ap(),
        "bo4": nc.dram_tensor("bo4", [1, D], mybir.dt.float8e4, kind="ExternalInput").ap(),
        "bgT": nc.dram_tensor("bgT", [128, KT], f32, kind="ExternalInput").ap(),
        "y": nc.dram_tensor("y", [L, D], bf16, kind="ExternalOutput").ap(),
    }
    with tile.TileContext(nc) as tc:
        with ExitStack() as ctx:
            _emit_main(nc, tile, mybir, ctx, tc, t)
    if SPLIT_WAITS:
        _split_multi_waits(nc, mybir)
    _CACHED["nc"] = nc
    return nc


def _pack(w):
    """[KT*128, C] -> [128, KT*C] with k-tiles side by side."""
    kt = w.shape[0] // 128
    return np.ascontiguousarray(
        w.reshape(kt, 128, w.shape[1]).transpose(1, 0, 2).reshape(128, -1))


F8 = ml_dtypes.float8_e4m3fn


def _hilo(a):
    hi = a.astype(F8)
    lo = (a - hi.astype(np.float32)).astype(F8)
    return hi, lo


def _prep_core_inputs(c, inputs, shared):
    b, g = c // 4, c % 4
    cols = slice(g * CW, (g + 1) * CW)
    bf_val = float(np.asarray(inputs["bf"]).reshape(-1)[0])
    wqh, wql = _hilo(_pack(inputs["Wq"][:, cols]) * WSCALE)
    wkh, wkl = _hilo(_pack(inputs["Wk"][:, cols]) * WSCALE)
    wvh, wvl = _hilo(_pack(inputs["Wv"][:, cols]) * WSCALE)
    m = {
        "xh": shared["xh"][b], "xl": shared["xl"][b],
        "wqh": wqh, "wql": wql, "wkh": wkh, "wkl": wkl,
        "wvh": wvh, "wvl": wvl,
        "wfb": shared["wfb"],
        "wo": _pack(inputs["Wo"][cols, :]).astype(BF16),
        "wg": shared["wg"],
        "ident": shared["ident"],
        "cb": np.concatenate([
            inputs["bq"][cols].reshape(2, 128).T,
            inputs["bk"][cols].reshape(2, 128).T,
            np.full((128, 1), bf_val, np.float32),
            np.broadcast_to(inputs["bv"][cols][None, :], (128, CW)),
        ], axis=1).astype(np.float32),
        "bo4": (inputs["bo"][None, :] * 0.25).astype(ml_dtypes.float8_e4m3fn),
        "bgT": (inputs["bg"].reshape(KT, 128).T * 0.25).astype(np.float32),
    }
    return m


def kernel(**inputs):
    from concourse import bass_utils

    nc = _build()
    xhl = [_hilo(_pack(inputs["x"][b].T)) for b in range(B)]
    shared = {
        "xh": [xhl[b][0] for b in range(B)],
        "xl": [xhl[b][1] for b in range(B)],
        "wg": _pack(inputs["Wg"]).astype(BF16),
        "wfb": np.ascontiguousarray(inputs["Wf"].reshape(KT, 128).T).astype(BF16),
        "ident": np.eye(128, dtype=np.float32).astype(BF16),
    }
    in_maps = [_prep_core_inputs(c, inputs, shared) for c in range(N_CORES)]
    res = bass_utils.run_bass_kernel_spmd(nc, in_maps, core_ids=list(range(N_CORES)))
    out = np.zeros((B, L, D), np.float32)
    for c in range(N_CORES):
        out[c // 4] += res.results[c]["y"]
    return out


# revision 7
# speedup vs baseline: 1.1152x; 1.0007x over previous
"""Trainium2 Bass kernel v2 for the gated-attention layer.

Sharding: 8 cores = (2 batches) x (4 head-groups of 4 heads each), as v1.

Key structure changes vs v1:
- Flipped AV matmul: out[q, head_dim] = ex_tile.T @ v_slice with N=65 per
  matmul (cost-model charges by output free size) -> AV cost halves, and
  denominators land per-partition (no DMA-bounce transposes).
- VG and gate pre-activations via N=1 matmuls (out [128,1] per k-tile).
- exp split between ScalarE (exact) and DVE (Schraudolph int16->bf16
  bitcast) so ACT is not the attention bottleneck.
- chunk-major loop (512 queries) with interleaved emission so PE never
  head-of-line blocks on exp.
- Output projection per chunk; y evacuated on ACT, DMA'd per 512-col tile.
"""

import sys

for _p in ("/root/.axon_site/_ro/trn_rl_repo", "/opt/trn_rl_repo"):
    if _p not in sys.path:
        sys.path.append(_p)

import numpy as np
import ml_dtypes

B, L, D, H = 2, 2048, 1024, 16
E = D // H          # 64 head dim
N_CORES = 8
HG = 4              # heads per core
CW = HG * E         # 256 cols per core
KT = D // 128       # 8 contraction k-tiles
NS = L // 128       # 16 key tiles
NCH = 4             # query chunks
CHW = L // NCH      # 512 queries per chunk
NQB = CHW // 128    # 4 q-blocks per chunk
SCALE = 1.0 / np.sqrt(E)

# Schraudolph exp on DVE for these key-tiles (rest on ACT); heads 0/1 are
# scored inside the tail window where DVE also runs norm/transpose-evac,
# so they get fewer DVE tiles.
DVE_KT_BY_H = {
    0: (1, 3, 5, 7, 9, 11, 13),
    1: (2, 4, 6, 8, 10, 12, 14),
    2: (0, 2, 4, 6, 8, 10, 12, 14, 15),
    3: (1, 3, 5, 7, 9, 11, 13, 15, 0),
}
SCH_A = 128.0 / np.log(2.0) * SCALE    # 23.083
WSCALE = 32.0   # qkv weights scaled up for fp8 hi-lo (denormal floor), undone at evac
SCH_B = 16250.5                        # 127*128 - 5.5 (centering) + 0.5 (trunc)

BF16 = ml_dtypes.bfloat16

_CACHED = {}
SPLIT_WAITS = True  # walrus HW build needs single-wait insts; CoreSim chokes on the split NoOps


def _patch_drain(tile_mod, mybir):
    """This walrus build only accepts one sync-wait on a Drain; spread the
    final Tile drain's waits over single-wait NOPs."""
    from concourse.vector_clock import ScopedClock

    def _dab(self, tick_clock, wait_clock):
        nc = self.nc
        drain_inst = nc.sync.drain()
        wait_clock.add_sem_waits(
            drain_inst.ins, ScopedClock({None: tick_clock.global_clock})
        )
        waits = list(drain_inst.ins.sync_info.on_wait)
        if len(waits) > 1:
            drain_inst.ins.sync_info.on_wait = waits[:1]
            for w in waits[1:]:
                nop = nc.sync.nop()
                if nop.ins.sync_info is None:
                    nop.ins.sync_info = mybir.SyncInfo(on_wait=[w], on_update=[])
                else:
                    nop.ins.sync_info.on_wait = [w]
        nc.all_engine_barrier()
        assert self.sems is not None
        popped = nc._tile_sem_poison_stack.pop()
        assert popped is self._sem_poison
        nc.clear_and_free_semaphores(list(self.sems.allocated().values()))
        nc.all_engine_barrier()

    tile_mod.TileContext._drain_and_barrier = _dab


def _split_multi_waits(nc, mybir):
    """One sync-wait per instruction; move extras onto same-engine NOPs."""
    ctr = 0
    for blk in nc.m.functions[0].blocks:
        insts = list(blk.instructions)
        out = []
        for inst in insts:
            si = getattr(inst, "sync_info", None)
            if si is not None and si.on_wait is not None and len(si.on_wait) > 1:
                waits = list(si.on_wait)
                for w in waits[:-1]:
                    nop = mybir.InstNoOp(
                        name=f"I-waitsplit-{ctr}",
                        engine=inst.engine,
                        sync_info=mybir.SyncInfo(on_wait=[w], on_update=[]),
                        bass_nofuse=True,
                    )
                    ctr += 1
                    out.append(nop)
                si.on_wait = waits[-1:]
            out.append(inst)
        if len(out) != len(insts):
            blk.instructions[:] = out


def _emit(nc, tile, mybir, ctx, tc, t):
    import concourse.bass as bass_mod

    f32 = mybir.dt.float32
    bf16 = mybir.dt.bfloat16
    i16 = mybir.dt.int16
    AF = mybir.ActivationFunctionType
    OP = mybir.AluOpType
    X = mybir.AxisListType.X

    consts = ctx.enter_context(tc.tile_pool(name="consts", bufs=1))
    dramp = ctx.enter_context(tc.tile_pool(name="dramp", bufs=2, space="DRAM"))

    # ---- SBUF constants / persistent tiles ----
    cb = consts.tile([128, 5 + CW], f32)
    nc.sync.dma_start(out=cb, in_=t["cb"])
    bq_t, bk_t = cb[:, 0:2], cb[:, 2:4]
    bf_c = cb[:, 4:5]
    bv_b = cb[:, 5:5 + CW]

    # all host-side tensors are packed [128, KT*cols] (k-tiles side by side);
    # x and qkv weights come as fp8 hi+lo pairs for DoubleRow hi-lo matmuls
    f8 = mybir.dt.float8e4
    xh_all = consts.tile([128, KT * L], f8, name="xh_all", tag="xh_all")
    xl_all = consts.tile([128, KT * L], f8, name="xl_all", tag="xl_all")
    xh = [xh_all[:, k * L:(k + 1) * L] for k in range(KT)]
    xl = [xl_all[:, k * L:(k + 1) * L] for k in range(KT)]

    def w_packed(name, cols, dt=bf16):
        all_t = consts.tile([128, KT * cols], dt, name=f"{name}_all",
                            tag=f"{name}_all")
        return all_t, [all_t[:, k * cols:(k + 1) * cols] for k in range(KT)]

    wqh_all, _ = w_packed("wqh", CW, f8)
    wql_all, _ = w_packed("wql", CW, f8)
    wkh_all, _ = w_packed("wkh", CW, f8)
    wkl_all, _ = w_packed("wkl", CW, f8)
    wvh_all, _ = w_packed("wvh", CW, f8)
    wvl_all, _ = w_packed("wvl", CW, f8)
    wg_all, wg = w_packed("wg", D)

    def dr3(big, off, mid, n):
        # [128, 2, n] DoubleRow AP into packed tile `big` at element offset
        return bass_mod.AP(tensor=big.tensor, offset=big.offset + off,
                           ap=[[big.ap[0][0], 128], [mid, 2], [1, n]])
    wfb = consts.tile([128, KT], bf16)
    ident = consts.tile([128, 128], bf16)
    bo4 = consts.tile([1, D], mybir.dt.float8e4)
    bgT = consts.tile([128, KT], f32)
    wo_all = consts.tile([128, 2 * D], bf16, name="wo_all", tag="wo_all")
    wo = [wo_all[:, k * D:(k + 1) * D] for k in range(2)]

    # few, large DMAs (HWDGE issue is a serialized ~630ns/DMA resource):
    # wv + wfb first, x hi/lo in 2-ktile pieces, then wq/wk, wo, wg.
    nc.sync.dma_start(out=wvh_all, in_=t["wvh"])
    nc.scalar.dma_start(out=wvl_all, in_=t["wvl"])
    nc.scalar.dma_start(out=wfb, in_=t["wfb"])

    def x_piece(sb_tile, dram_ap, lc):
        # all KT k-tiles, L-columns [lc*512, (lc+1)*512): lets the s-loop
        # start after the first piece instead of after the whole tensor
        mk = lambda ap, off: bass_mod.AP(
            tensor=ap.tensor, offset=ap.offset + off,
            ap=[[ap.ap[0][0], 128], [L, KT], [1, 512]])
        return mk(sb_tile, lc * 512), mk(dram_ap, lc * 512)

    _wdmas = [("wqh", wqh_all), ("wql", wql_all), ("wkh", wkh_all),
              ("wkl", wkl_all)]
    for lc in range(4):
        o, i = x_piece(xh_all, t["xh"], lc)
        nc.sync.dma_start(out=o, in_=i)
        o, i = x_piece(xl_all, t["xl"], lc)
        nc.scalar.dma_start(out=o, in_=i)
        if lc < 2:
            for nm, dst_t in _wdmas[2 * lc:2 * lc + 2]:
                nc.sync.dma_start(out=dst_t, in_=t[nm])
    nc.sync.dma_start(out=wo_all, in_=t["wo"])
    nc.scalar.dma_start(out=ident, in_=t["ident"])
    nc.scalar.dma_start(out=bo4, in_=t["bo4"])
    nc.scalar.dma_start(out=bgT, in_=t["bgT"])
    for half in range(2):
        nc.sync.dma_start(out=wg_all[:, half * 4 * D:(half + 1) * 4 * D],
                          in_=t["wg"][:, half * 4 * D:(half + 1) * 4 * D])

    qt = [consts.tile([128, L], bf16, name=f"qt{i}", tag=f"qt{i}") for i in range(2)]
    kt_ = [consts.tile([128, L], bf16, name=f"kt{i}", tag=f"kt{i}") for i in range(2)]
    v_aug = [consts.tile([128, CW], bf16, name=f"va{i}", tag=f"va{i}")
             for i in range(NS)]
    ones_c = consts.tile([128, 1], bf16)
    vt = [consts.tile([128, L], bf16, name=f"vt{i}", tag=f"vt{i}") for i in range(2)]
    gate_t = consts.tile([128, NS], f32)
    gate_b = consts.tile([128, NS], mybir.dt.float8e4)
    omg_b = consts.tile([128, NS], mybir.dt.float8e4)
    gateomg = consts.tile([1, 2 * L], mybir.dt.float8e4)
    bovg = consts.tile([1, 2 * D], mybir.dt.float8e4)
    xsum = consts.tile([128, KT], f32)
    xsum_b = consts.tile([128, KT], bf16)
    vg4T = consts.tile([128, KT], mybir.dt.float8e4)

    nc.vector.memset(ones_c, 1.0)

    # ---- phase A: projections, gate, global context ----
    with tc.tile_pool(name="pav", bufs=2, space="PSUM") as pav, \
         tc.tile_pool(name="paq", bufs=2, space="PSUM") as paq, \
         tc.tile_pool(name="pag", bufs=1, space="PSUM") as pag:
        # V projection + per-tile evac with bias, interleaved with the
        # first c-slot's Q/K blocks so PE fills x-piece DMA waits
        NKP = KT // 2
        DR = mybir.MatmulPerfMode.DoubleRow

        def v_block(s):
            ps = pav.tile([128, CW], f32, name="vps", tag="vps")
            terms = ((xh_all, wvh_all), (xh_all, wvl_all), (xl_all, wvh_all))
            for ti, (xa, wa) in enumerate(terms):
                for kp in range(NKP):
                    nc.tensor.matmul(
                        out=ps,
                        lhsT=dr3(xa, 2 * kp * L + s * 128, L, 128),
                        rhs=dr3(wa, 2 * kp * CW, CW, CW),
                        start=(ti == 0 and kp == 0),
                        stop=(ti == 2 and kp == NKP - 1), perf_mode=DR)
            nc.vector.scalar_tensor_tensor(out=v_aug[s], in0=ps,
                                           scalar=1.0 / WSCALE, op0=OP.mult,
                                           in1=bv_b, op1=OP.add)

        def qk_block(c, proj, lo):
            dst, wh, wl, bias = ((qt, wqh_all, wql_all, bq_t),
                                 (kt_, wkh_all, wkl_all, bk_t))[proj]
            ps = paq.tile([128, 512], f32, name="qkps", tag="qkps")
            terms = ((wh, xh_all), (wl, xh_all), (wh, xl_all))
            for ti, (wa, xa) in enumerate(terms):
                for kp in range(NKP):
                    nc.tensor.matmul(
                        out=ps,
                        lhsT=dr3(wa, 2 * kp * CW + c * 128, CW, 128),
                        rhs=dr3(xa, 2 * kp * L + lo, L, 512),
                        start=(ti == 0 and kp == 0),
                        stop=(ti == 2 and kp == NKP - 1), perf_mode=DR)
            if lo % 1024 == 0:
                nc.scalar.activation(dst[c][:, lo:lo + 512], ps,
                                     AF.Identity, bias=bias[:, c:c + 1],
                                     scale=1.0 / WSCALE)
            else:
                nc.vector.tensor_scalar(
                    out=dst[c][:, lo:lo + 512], in0=ps,
                    scalar1=1.0 / WSCALE, scalar2=bias[:, c:c + 1],
                    op0=OP.mult, op1=OP.add)

        for p in range(4):
            for s in range(4 * p, 4 * p + 4):
                v_block(s)
            qk_block(0, 0, 512 * p)
            qk_block(0, 1, 512 * p)
        # gate pre-activations via N=1 matmuls, one sigmoid
        gpre = pag.tile([128, NS], f32, name="gpre", tag="gpre")
        for s in range(NS):
            for xi, xa in enumerate((xh_all, xl_all)):
                for k in range(KT):
                    nc.tensor.matmul(
                        out=gpre[:, s:s + 1],
                        lhsT=bass_mod.AP(tensor=xa.tensor,
                                         offset=xa.offset + k * L + s * 128,
                                         ap=[[xa.ap[0][0], 128], [1, 128]]),
                        rhs=wfb[:, k:k + 1],
                        start=(xi == 0 and k == 0),
                        stop=(xi == 1 and k == KT - 1))
        nc.scalar.activation(gate_t, gpre, AF.Sigmoid, bias=bf_c)
        nc.vector.tensor_copy(gate_b, gate_t)
        nc.vector.tensor_scalar(out=omg_b, in0=gate_t, scalar1=-1.0, scalar2=1.0,
                                op0=OP.mult, op1=OP.add)
        gd = dramp.tile([1, L], mybir.dt.float8e4, name="gd", tag="gd")
        od = dramp.tile([1, L], mybir.dt.float8e4, name="od", tag="od")
        nc.sync.dma_start(out=gd, in_=gate_b)
        nc.sync.dma_start(out=od, in_=omg_b)
        nc.sync.dma_start(out=gateomg[:, 0:L], in_=bass_mod.AP(
            tensor=gd.tensor, offset=gd.offset, ap=[[0, 1], [1, NS], [NS, 128]]))
        nc.sync.dma_start(out=gateomg[:, L:2 * L], in_=bass_mod.AP(
            tensor=od.tensor, offset=od.offset, ap=[[0, 1], [1, NS], [NS, 128]]))
        nc.sync.dma_start(out=bovg[:, 0:D], in_=bo4)

        # second c-slot Q/K projections
        for proj in range(2):
            for lo in range(0, L, 512):
                qk_block(1, proj, lo)

        # (global-context xsum/VG emitted later, inside chunk 0: emit_vg)

    # ---- phase B/C: attention + output projection, chunk-major ----
    ex_sb = ctx.enter_context(tc.tile_pool(name="ex_sb", bufs=2))
    avn_sb = ctx.enter_context(tc.tile_pool(name="avn_sb", bufs=3))
    rc_sb = ctx.enter_context(tc.tile_pool(name="rc_sb", bufs=2))
    y_sb = ctx.enter_context(tc.tile_pool(name="y_sb", bufs=3))
    # single PSUM pool, per-tag bufs: st x2 + avs x4 + tr x1 + yps x1 = 8 banks
    psb = ctx.enter_context(tc.tile_pool(name="psb", bufs=1, space="PSUM"))
    stp = avp = trp = yp = psb

    ex = {}     # (h) -> list of 16 ex tiles for current chunk
    avs = {}    # qb -> psum tile [128, HG*(E+1)]
    yd = t["y"].rearrange("(t p) d -> t p d", p=128)

    def emit_vg():
        xsum_scratch = consts.tile([128, L], bf16, name="xsum_scratch")
        xsuml = consts.tile([128, KT], f32, name="xsuml")
        for k in range(KT):
            nc.scalar.activation(xsum_scratch, xh[k], AF.Copy,
                                 accum_out=xsum[:, k:k + 1])
            nc.vector.reduce_sum(out=xsuml[:, k:k + 1], in_=xl[k], axis=X)
        nc.vector.tensor_add(xsum, xsum, xsuml)
        nc.vector.tensor_copy(xsum_b, xsum)
        vgT = yp.tile([128, KT], f32, name="vgT", tag="tail", bufs=2)
        for ch in range(KT):
            for k in range(KT):
                nc.tensor.matmul(out=vgT[:, ch:ch + 1],
                                 lhsT=wg[k][:, ch * 128:(ch + 1) * 128],
                                 rhs=xsum_b[:, k:k + 1],
                                 start=(k == 0), stop=(k == KT - 1))
        nc.vector.scalar_tensor_tensor(out=vg4T, in0=vgT, scalar=0.25 / L,
                                       op0=OP.mult, in1=bgT, op1=OP.add)
        vgd = dramp.tile([1, D], mybir.dt.float8e4, name="vgd", tag="vgd")
        nc.sync.dma_start(out=vgd, in_=vg4T)
        nc.sync.dma_start(out=bovg[:, D:2 * D], in_=bass_mod.AP(
            tensor=vgd.tensor, offset=vgd.offset, ap=[[0, 1], [1, KT], [KT, 128]]))

    def emit_scores(c, h):
        dve_set = DVE_KT_BY_H[h]
        exl = []
        for s in range(NS):
            st = stp.tile([128, CHW], f32, name="st", tag="st", bufs=3)
            nc.tensor.matmul(
                out=st,
                lhsT=kt_[h // 2][64 * (h % 2):64 * (h % 2) + 64, s * 128:(s + 1) * 128],
                rhs=qt[h // 2][64 * (h % 2):64 * (h % 2) + 64, c * CHW:(c + 1) * CHW],
                start=True, stop=True)
            ext = ex_sb.tile([128, CHW], bf16, name="ex", tag=f"ex{s}", bufs=3)
            if s in dve_set:
                nc.vector.tensor_scalar(out=ext.bitcast(i16), in0=st,
                                        scalar1=SCH_A, scalar2=SCH_B,
                                        op0=OP.mult, op1=OP.add)
            else:
                nc.scalar.activation(ext, st, AF.Exp, scale=SCALE)
            exl.append(ext)
        ex[h] = exl

    def emit_av(c, h):
        if h == 0:
            avs[0] = avp.tile([128, 2 * CW], f32, name="avsA", tag="avsA")
            avs[1] = avp.tile([128, 2 * CW], f32, name="avsB", tag="avsB")
            avs["d"] = avp.tile([128, HG * NQB], f32, name="avsd", tag="avsd")
        for qb in range(NQB):
            lo = (qb % 2) * CW + h * E
            dcol = h * NQB + qb
            for s in range(NS):
                exsl = ex[h][s][:, qb * 128:(qb + 1) * 128]
                nc.tensor.matmul(
                    out=avs[qb // 2][:, lo:lo + E],
                    lhsT=exsl, rhs=v_aug[s][:, h * E:(h + 1) * E],
                    start=(s == 0), stop=(s == NS - 1))
                nc.tensor.matmul(
                    out=avs["d"][:, dcol:dcol + 1],
                    lhsT=exsl, rhs=ones_c,
                    start=(s == 0), stop=(s == NS - 1))

    def emit_norm(c, qbs=None):
        # normalize + transpose into vt for chunk c
        for qb in (range(NQB) if qbs is None else qbs):
            lt = c * NQB + qb
            d4 = avs["d"].rearrange("p (h q) -> p h q", q=NQB)[:, :, qb]
            rc4 = rc_sb.tile([128, HG], f32, name="rc4", tag="rc4")
            nc.vector.reciprocal(rc4, d4)
            nc.vector.tensor_scalar(out=rc4, in0=rc4,
                                    scalar1=gate_t[:, lt:lt + 1], scalar2=None,
                                    op0=OP.mult)
            avn = avn_sb.tile([128, HG * E], bf16, name="avn", tag="avn")
            rc_b = bass_mod.AP(tensor=rc4.tensor, offset=rc4.offset,
                               ap=[[rc4.ap[0][0], 128], [1, HG], [0, E]])
            src_ap = avs[qb // 2][:, (qb % 2) * CW:(qb % 2) * CW + CW]
            nc.vector.tensor_tensor(
                out=avn.rearrange("p (h c) -> p h c", c=E),
                in0=src_ap.rearrange("p (h c) -> p h c", c=E),
                in1=rc_b, op=OP.mult)
            for kc in range(2):
                tr = trp.tile([128, 128], bf16, name="tr", tag="tail", bufs=2)
                nc.tensor.transpose(tr, avn[:, kc * 128:(kc + 1) * 128], ident)
                nc.scalar.activation(vt[kc][:, lt * 128:(lt + 1) * 128], tr,
                                     AF.Copy)

    def emit_wo(c, qbs=None):
        # output projection + bias fusion + evac + DMA for chunk c
        for qb in (range(NQB) if qbs is None else qbs):
            lt = c * NQB + qb
            lsl = slice(lt * 128, (lt + 1) * 128)
            for do in range(0, D, 512):
                ps = yp.tile([128, 512], f32, name="yps", tag="tail", bufs=2)
                nc.tensor.matmul(out=ps, lhsT=vt[0][:, lsl],
                                 rhs=wo[0][:, do:do + 512], start=True, stop=False)
                nc.tensor.matmul(out=ps, lhsT=vt[1][:, lsl],
                                 rhs=wo[1][:, do:do + 512], start=False, stop=False)
                gdr = bass_mod.AP(tensor=gateomg.tensor, offset=gateomg.offset
                                  + lt * 128, ap=[[gateomg.ap[0][0], 1], [L, 2],
                                                  [1, 128]])
                bdr = bass_mod.AP(tensor=bovg.tensor, offset=bovg.offset + do,
                                  ap=[[bovg.ap[0][0], 1], [D, 2], [1, 512]])
                nc.tensor.matmul(out=ps, lhsT=gdr, rhs=bdr, start=False,
                                 stop=True, perf_mode=mybir.MatmulPerfMode.DoubleRow)
                ot = y_sb.tile([128, 512], bf16, name="ot", tag="ot")
                if do == 0:
                    nc.scalar.activation(ot, ps, AF.Copy)
                else:
                    nc.vector.tensor_copy(ot, ps)
                nc.sync.dma_start(out=yd[lt][:, do:do + 512], in_=ot)

    # interleaved emission: scores run ahead of AV; each chunk's norm and
    # output projection slot between the next chunk's first score blocks so
    # no engine head-of-line blocks.
    for c in range(NCH):
        if c == 0:
            emit_scores(0, 0)
            emit_scores(0, 1)
        emit_av(c, 0)
        if c == 0:
            emit_vg()
        emit_scores(c, 2)
        emit_av(c, 1)
        emit_scores(c, 3)
        emit_av(c, 2)
        if c + 1 < NCH:
            emit_scores(c + 1, 0)
        emit_av(c, 3)
        if c + 1 < NCH:
            emit_scores(c + 1, 1)
        for qb in range(NQB):
            emit_norm(c, (qb,))
            emit_wo(c, (qb,))


def _emit_main(nc, tile, mybir, ctx, tc, t):
    _emit(nc, tile, mybir, ctx, tc, t)


def _build():
    if "nc" in _CACHED:
        return _CACHED["nc"]
    import concourse.bass as bass
    import concourse.tile as tile
    from concourse import mybir
    from contextlib import ExitStack

    _patch_drain(tile, mybir)
    nc = bass.Bass("TRN2", target_bir_lowering=False, debug=False)
    f32, bf16 = mybir.dt.float32, mybir.dt.bfloat16
    t = {
        "xh": nc.dram_tensor("xh", [128, KT * L], mybir.dt.float8e4, kind="ExternalInput").ap(),
        "xl": nc.dram_tensor("xl", [128, KT * L], mybir.dt.float8e4, kind="ExternalInput").ap(),
        "cb": nc.dram_tensor("cb", [128, 5 + CW], f32, kind="ExternalInput").ap(),
        "wqh": nc.dram_tensor("wqh", [128, KT * CW], mybir.dt.float8e4, kind="ExternalInput").ap(),
        "wql": nc.dram_tensor("wql", [128, KT * CW], mybir.dt.float8e4, kind="ExternalInput").ap(),
        "wkh": nc.dram_tensor("wkh", [128, KT * CW], mybir.dt.float8e4, kind="ExternalInput").ap(),
        "wkl": nc.dram_tensor("wkl", [128, KT * CW], mybir.dt.float8e4, kind="ExternalInput").ap(),
        "wvh": nc.dram_tensor("wvh", [128, KT * CW], mybir.dt.float8e4, kind="ExternalInput").ap(),
        "wvl": nc.dram_tensor("wvl", [128, KT * CW], mybir.dt.float8e4, kind="ExternalInput").ap(),
        "wfb": nc.dram_tensor("wfb", [128, KT], bf16, kind="ExternalInput").ap(),
        "wo": nc.dram_tensor("wo", [128, 2 * D], bf16, kind="ExternalInput").ap(),
        "wg": nc.dram_tensor("wg", [128, KT * D], bf16, kind="ExternalInput").ap(),
        "ident": nc.dram_tensor("ident", [128, 128], bf16, kind="ExternalInput").ap(),
        "bo4": nc.dram_tensor("bo4", [1, D], mybir.dt.float8e4, kind="ExternalInput").ap(),
        "bgT": nc.dram_tensor("bgT", [128, KT], f32, kind="ExternalInput").ap(),
        "y": nc.dram_tensor("y", [L, D], bf16, kind="ExternalOutput").ap(),
    }
    with tile.TileContext(nc) as tc:
        with ExitStack() as ctx:
            _emit_main(nc, tile, mybir, ctx, tc, t)
    if SPLIT_WAITS:
        _split_multi_waits(nc, mybir)
    _CACHED["nc"] = nc
    return nc


def _pack(w):
    """[KT*128, C] -> [128, KT*C] with k-tiles side by side."""
    kt = w.shape[0] // 128
    return np.ascontiguousarray(
        w.reshape(kt, 128, w.shape[1]).transpose(1, 0, 2).reshape(128, -1))


F8 = ml_dtypes.float8_e4m3fn


def _hilo(a):
    hi = a.astype(F8)
    lo = (a - hi.astype(np.float32)).astype(F8)
    return hi, lo


def _prep_core_inputs(c, inputs, shared):
    b, g = c // 4, c % 4
    cols = slice(g * CW, (g + 1) * CW)
    bf_val = float(np.asarray(inputs["bf"]).reshape(-1)[0])
    wqh, wql = _hilo(_pack(inputs["Wq"][:, cols]) * WSCALE)
    wkh, wkl = _hilo(_pack(inputs["Wk"][:, cols]) * WSCALE)
    wvh, wvl = _hilo(_pack(inputs["Wv"][:, cols]) * WSCALE)
    m = {
        "xh": shared["xh"][b], "xl": shared["xl"][b],
        "wqh": wqh, "wql": wql, "wkh": wkh, "wkl": wkl,
        "wvh": wvh, "wvl": wvl,
        "wfb": shared["wfb"],
        "wo": _pack(inputs["Wo"][cols, :]).astype(BF16),
        "wg": shared["wg"],
        "ident": shared["ident"],
        "cb": np.concatenate([
            inputs["bq"][cols].reshape(2, 128).T,
            inputs["bk"][cols].reshape(2, 128).T,
            np.full((128, 1), bf_val, np.float32),
            np.broadcast_to(inputs["bv"][cols][None, :], (128, CW)),
        ], axis=1).astype(np.float32),
        "bo4": (inputs["bo"][None, :] * 0.25).astype(ml_dtypes.float8_e4m3fn),
        "bgT": (inputs["bg"].reshape(KT, 128).T * 0.25).astype(np.float32),
    }
    return m


def kernel(**inputs):
    from concourse import bass_utils

    nc = _build()
    xhl = [_hilo(_pack(inputs["x"][b].T)) for b in range(B)]
    shared = {
        "xh": [xhl[b][0] for b in range(B)],
        "xl": [xhl[b][1] for b in range(B)],
        "wg": _pack(inputs["Wg"]).astype(BF16),
        "wfb": np.ascontiguousarray(inputs["Wf"].reshape(KT, 128).T).astype(BF16),
        "ident": np.eye(128, dtype=np.float32).astype(BF16),
    }
    in_maps = [_prep_core_inputs(c, inputs, shared) for c in range(N_CORES)]
    res = bass_utils.run_bass_kernel_spmd(nc, in_maps, core_ids=list(range(N_CORES)))
    out = np.zeros((B, L, D), np.float32)
    for c in range(N_CORES):
        out[c // 4] += res.results[c]["y"]
    return out
